# revision 1
# baseline (speedup 1.0000x reference)
"""DCGRU cell on 8 Trainium2 NeuronCores.

Strategy (dst-sharded graph partitioning):
  - Nodes are sharded into 8 contiguous ranges (one per core). Edges are
    assigned to the core owning dst; within a core, dst nodes are processed
    in blocks of 128, grouped into super-blocks of SB_BLOCKS for gathering.
  - Because dma_gather takes int16 indices, the gather tables are split in
    two halves (src < N/2 and src >= N/2); per (block, half) the edges are
    padded to groups of 128 (group counts uniform across cores so a single
    SPMD instruction stream works). One dma_gather covers a whole
    (super-block, half) run of groups.
  - Pass 1 gathers x1 = [feat, state] rows (bf16, 256B) and segment-sums
    them via one-hot matmuls into PSUM (transposed layout), then
    zr = sigmoid(aggT.T @ Wzr + bzr); rs = r * state.
  - y2 = feat @ Wc_top + rs @ Wc_bot is computed per block (bf16) and
    AllGathered across cores (6.4MB). Pass 2 gathers 256B rows each holding
    a PAIR of adjacent nodes' y2 vectors (index src_pos//2); a single
    [128, 256] "paired one-hot" per group (iota vs dst_local + 128*parity)
    feeds two matmuls that select the correct half, giving
    c = tanh(agg + bc); new_state = z*state + (1-z)*c.
"""

import numpy as np

import concourse.bass as bass
import concourse.bacc as bacc
import concourse.mybir as mybir
import concourse.tile as tile
from concourse.bass_utils import run_bass_kernel_spmd
from concourse.library_config import mlp
from concourse.masks import make_identity

N_NODES = 50000
N_EDGES = 640000
HID = 64
N_CORES = 8
BLK = 128           # dst nodes per block (= PSUM partition dim)
SB_BLOCKS = 1       # dst blocks per super-block (gather instruction scope)
MAX_G_CHUNK = 12    # cap on groups per dma_gather instruction (pass 1)
MAX_GC_CHUNK = 20   # cap on groups per dma_gather instruction (pass 2)

F32 = mybir.dt.float32
BF16 = mybir.dt.bfloat16
I16 = mybir.dt.int16


def _prep_edges(dst, src, edge_weight, n_nodes, n_cores):
    """Partition edges by dst core/block and src half; build per-core arrays.

    Group order: for each super-block, all lo-half groups of its blocks,
    then all hi-half groups.

    Returns (idx16, dst_t, w_t, plan) where plan is a dict with
      chunks:   list of (g0, g1, cls) gather chunk spans
      blk_of_g: block id of each group
      first_g, last_g: per block, first/last group id
      nblk, ngroups
    """
    shard = n_nodes // n_cores
    nblk = (shard + BLK - 1) // BLK
    split = n_nodes // 2
    e = len(dst)

    dsts = dst.astype(np.int64)
    srcs = src.astype(np.int64)
    owner = dsts // shard
    local = dsts - owner * shard

    # Balance in-degree across blocks: per core, deal nodes (sorted by
    # in-degree, desc) round-robin over blocks. pos[core, orig_local] is the
    # node's new row; node tables / shard rows / outputs use this order.
    deg = np.zeros(n_nodes, np.int64)
    np.add.at(deg, dsts, 1)
    pos = np.empty((n_cores, shard), np.int64)
    blk_fill = np.empty(nblk, np.int64)
    cap = np.full(nblk, BLK, np.int64)
    cap[nblk - 1] = shard - (nblk - 1) * BLK
    for p in range(n_cores):
        nodes = np.argsort(-deg[p * shard : (p + 1) * shard], kind="stable")
        blk_fill[:] = 0
        bi = 0
        for n in nodes:
            while blk_fill[bi % nblk] >= cap[bi % nblk]:
                bi += 1
            b = bi % nblk
            pos[p, n] = b * BLK + blk_fill[b]
            blk_fill[b] += 1
            bi += 1
    newloc = pos[owner, local]
    b_of = newloc // BLK
    local = newloc
    cls = (srcs >= split).astype(np.int64)

    cnt = np.zeros((n_cores, nblk, 2), np.int64)
    np.add.at(cnt, (owner, b_of, cls), 1)
    gpbc = -(-cnt.max(axis=0) // BLK)  # [nblk, 2] groups per (block, half)
    # ensure every block has at least one group so its PSUM accum is defined
    empty = gpbc.sum(axis=1) == 0
    gpbc[empty, 0] = 1

    # unit order: (super-block, class, block)
    unit_order = []
    for sb0 in range(0, nblk, SB_BLOCKS):
        sbb = range(sb0, min(sb0 + SB_BLOCKS, nblk))
        for c in range(2):
            for b in sbb:
                unit_order.append((b, c))
    unit_sizes = np.array([gpbc[b, c] for b, c in unit_order], np.int64)
    unit_off = np.concatenate([[0], np.cumsum(unit_sizes)])
    ngroups = int(unit_off[-1])
    unit_idx = {bc: i for i, bc in enumerate(unit_order)}

    # per-group block id and per-block first/last group
    blk_of_g = np.zeros(ngroups, np.int64)
    for i, (b, c) in enumerate(unit_order):
        blk_of_g[unit_off[i] : unit_off[i + 1]] = b
    first_g = np.full(nblk, -1, np.int64)
    last_g = np.full(nblk, -1, np.int64)
    for g in range(ngroups):
        b = blk_of_g[g]
        if first_g[b] < 0:
            first_g[b] = g
        last_g[b] = g

    # gather chunks: contiguous same-class unit runs within a super-block
    chunks = []
    i = 0
    while i < len(unit_order):
        c = unit_order[i][1]
        sb = unit_order[i][0] // SB_BLOCKS
        j = i
        while (
            j < len(unit_order)
            and unit_order[j][1] == c
            and unit_order[j][0] // SB_BLOCKS == sb
        ):
            j += 1
        g0, g1 = int(unit_off[i]), int(unit_off[j])
        for s in range(g0, g1, MAX_G_CHUNK):
            if s < g1:
                chunks.append((s, min(s + MAX_G_CHUNK, g1), c))
        i = j

    # slot assignment
    ukey = np.array([unit_idx[(b, c)] for b, c in zip(b_of, cls)], np.int64)
    order = np.argsort(ukey, kind="stable")
    ukey_s = ukey[order]
    owner_s = owner[order]
    srcs_s = srcs[order]
    cls_s = cls[order]
    dloc_s = (local % BLK)[order].astype(np.float32)
    ws_s = edge_weight.astype(np.float32)[order]
    src_pos = pos[srcs_s // shard, srcs_s % shard] + (srcs_s // shard) * shard
    src_local = src_pos - cls_s * split

    # rank within (core, unit)
    ck = owner_s * len(unit_order) + ukey_s
    order2 = np.argsort(ck, kind="stable")
    ck = ck[order2]
    owner_s = owner_s[order2]
    src_local = src_local[order2]
    ukey_s = ukey_s[order2]
    dloc_s = dloc_s[order2]
    ws_s = ws_s[order2]
    cls_sorted2 = cls_s[order2]
    bucket_start = np.searchsorted(ck, np.arange(n_cores * len(unit_order)))
    rank = np.arange(e) - bucket_start[ck]
    g_global = unit_off[ukey_s] + rank // BLK
    lane = rank % BLK

    # recover permuted global src position for pass-2 pair indexing
    src_posg = src_local + cls_sorted2 * split
    par = (src_posg % 2).astype(np.float32)
    pair_idx = src_posg // 2

    idx16 = np.zeros((n_cores, 16, 8 * ngroups), np.int16)
    idx16p = np.zeros((n_cores, 16, 8 * ngroups), np.int16)
    dst_t = np.zeros((n_cores, BLK, ngroups), np.float32)
    dstp_t = np.zeros((n_cores, BLK, ngroups), np.float32)
    w_t = np.zeros((n_cores, BLK, ngroups), np.float32)
    idx16[owner_s, lane % 16, 8 * g_global + lane // 16] = src_local.astype(np.int16)
    idx16p[owner_s, lane % 16, 8 * g_global + lane // 16] = pair_idx.astype(np.int16)
    dst_t[owner_s, lane, g_global] = dloc_s
    dstp_t[owner_s, lane, g_global] = dloc_s + BLK * par
    w_t[owner_s, lane, g_global] = ws_s
    idx16 = np.tile(idx16, (1, 8, 1))
    idx16p = np.tile(idx16p, (1, 8, 1))

    # phase-C chunks: pass 2 has a single gather table, so each block's
    # whole group span is one chunk (split only by the SBUF tile cap)
    chunks_c = []
    for b in range(nblk):
        ga, gb = int(first_g[b]), int(last_g[b]) + 1
        for s in range(ga, gb, MAX_GC_CHUNK):
            chunks_c.append((s, min(s + MAX_GC_CHUNK, gb), 0))

    plan = {
        "chunks": chunks,
        "chunks_c": chunks_c,
        "blk_of_g": [int(x) for x in blk_of_g],
        "first_g": [int(x) for x in first_g],
        "last_g": [int(x) for x in last_g],
        "nblk": nblk,
        "ngroups": ngroups,
        "pos": pos,
    }
    return idx16, idx16p, dst_t, dstp_t, w_t, plan


def _build(n_nodes, hid, plan, n_cores, n_queues=4):
    """Build the SPMD Bass program from the edge plan."""
    shard = n_nodes // n_cores
    nblk = plan["nblk"]
    ngroups = plan["ngroups"]
    chunks = plan["chunks"]
    blk_of_g = plan["blk_of_g"]
    first_g = plan["first_g"]
    last_g = plan["last_g"]
    split = n_nodes // 2
    h2 = 2 * hid

    nc = bacc.Bacc(None, num_devices=n_cores, num_swdge_queues=n_queues)

    x1b = nc.dram_tensor("x1b", [n_nodes, h2], BF16, kind="ExternalInput")
    feat_s = nc.dram_tensor("feat_s", [shard, hid], F32, kind="ExternalInput")
    state_s = nc.dram_tensor("state_s", [shard, hid], F32, kind="ExternalInput")
    idx16_d = nc.dram_tensor("idx16", [BLK, 8 * ngroups], I16, kind="ExternalInput")
    idx16p_d = nc.dram_tensor("idx16p", [BLK, 8 * ngroups], I16, kind="ExternalInput")
    dst_d = nc.dram_tensor("dst_t", [BLK, ngroups], F32, kind="ExternalInput")
    dstp_d = nc.dram_tensor("dstp_t", [BLK, ngroups], F32, kind="ExternalInput")
    w_d = nc.dram_tensor("w_t", [BLK, ngroups], F32, kind="ExternalInput")
    wzr = nc.dram_tensor("wzr", [h2, h2], F32, kind="ExternalInput")
    bzr = nc.dram_tensor("bzr", [1, h2], F32, kind="ExternalInput")
    wc = nc.dram_tensor("wc", [h2, hid], F32, kind="ExternalInput")
    bc = nc.dram_tensor("bc", [1, hid], F32, kind="ExternalInput")
    out = nc.dram_tensor("out", [shard, hid], F32, kind="ExternalOutput")

    y2_shard = nc.dram_tensor("y2_shard", [shard, hid], BF16, kind="Internal")
    y2_full = nc.dram_tensor(
        "y2_full", [n_nodes, hid], BF16, kind="Internal", addr_space="Shared"
    )

    qn = [0]

    def next_q():
        q = qn[0]
        qn[0] = (qn[0] + 1) % n_queues
        return q

    def rows_of(b):
        return BLK if b < nblk - 1 else shard - (nblk - 1) * BLK

    with tile.TileContext(nc) as tc:
        with (
            tc.tile_pool(name="const", bufs=1) as const_pool,
            tc.tile_pool(name="store", bufs=1) as store_pool,
            tc.tile_pool(name="msg", bufs=4) as msg_pool,
            tc.tile_pool(name="oh", bufs=6) as oh_pool,
            tc.tile_pool(name="blk", bufs=5) as blk_pool,
            tc.tile_pool(name="agg_ps", bufs=5, space="PSUM") as agg_psum,
            tc.tile_pool(name="mm_ps", bufs=3, space="PSUM") as mm_psum,
        ):
            nc.gpsimd.load_library(mlp)
            # ---- constants ----
            iota_i = const_pool.tile([BLK, BLK], mybir.dt.int32)
            nc.gpsimd.iota(iota_i[:], pattern=[[1, BLK]], base=0, channel_multiplier=0)
            iota_f = const_pool.tile([BLK, BLK], F32)
            nc.vector.tensor_copy(iota_f[:], iota_i[:])
            iota_h = const_pool.tile([BLK, BLK], BF16)
            nc.vector.tensor_copy(iota_h[:], iota_i[:])
            iota2_i = const_pool.tile([BLK, 2 * BLK], mybir.dt.int32)
            nc.gpsimd.iota(
                iota2_i[:], pattern=[[1, 2 * BLK]], base=0, channel_multiplier=0
            )
            iota2_h = const_pool.tile([BLK, 2 * BLK], BF16)
            nc.vector.tensor_copy(iota2_h[:], iota2_i[:])
            identity = const_pool.tile([BLK, BLK], F32)
            make_identity(nc, identity[:])
            ones1 = const_pool.tile([1, BLK], F32)
            nc.vector.memset(ones1[:], 1.0)
            wzr_sb = const_pool.tile([h2, h2], F32)
            nc.sync.dma_start(out=wzr_sb[:], in_=wzr[:, :])
            bzr_sb = const_pool.tile([1, h2], F32)
            nc.sync.dma_start(out=bzr_sb[:], in_=bzr[:, :])
            wctop_sb = const_pool.tile([hid, hid], F32)
            nc.sync.dma_start(out=wctop_sb[:], in_=wc[0:hid, :])
            wcbot_sb = const_pool.tile([hid, hid], F32)
            nc.sync.dma_start(out=wcbot_sb[:], in_=wc[hid:h2, :])
            bc_sb = const_pool.tile([1, hid], F32)
            nc.sync.dma_start(out=bc_sb[:], in_=bc[:, :])

            # ---- persistent stores (indices/weights loaded once) ----
            idx16_sb = store_pool.tile([BLK, 8 * ngroups], I16)
            nc.sync.dma_start(out=idx16_sb[:], in_=idx16_d[:, :])
            idx16p_sb = store_pool.tile([BLK, 8 * ngroups], I16)
            nc.sync.dma_start(out=idx16p_sb[:], in_=idx16p_d[:, :])
            dst_sb = store_pool.tile([BLK, ngroups], F32)
            nc.sync.dma_start(out=dst_sb[:], in_=dst_d[:, :])
            w_sb = store_pool.tile([BLK, ngroups], F32)
            nc.sync.dma_start(out=w_sb[:], in_=w_d[:, :])
            dstp_sb = store_pool.tile([BLK, ngroups], F32)
            nc.sync.dma_start(out=dstp_sb[:], in_=dstp_d[:, :])
            z_store = store_pool.tile([BLK, nblk * hid], F32)
            st_store = store_pool.tile([BLK, nblk * hid], F32)
            nc.vector.memset(z_store[:], 0.0)
            nc.vector.memset(st_store[:], 0.0)

            # ============== Phase A: pass-1 aggregation + y2 ===============
            psum_of = {}

            def tail_a(b):
                """Post-aggregation per-block work for pass 1."""
                R = rows_of(b)
                aggT_sb = blk_pool.tile([h2, BLK], F32, tag="aggT")
                nc.vector.tensor_copy(aggT_sb[:], psum_of.pop(b)[:])
                zr_ps = mm_psum.tile([BLK, h2], F32, tag="mm")
                nc.tensor.matmul(
                    zr_ps[:], lhsT=aggT_sb[:], rhs=wzr_sb[:], start=True, stop=False
                )
                nc.tensor.matmul(
                    zr_ps[:], lhsT=ones1[:], rhs=bzr_sb[:], start=False, stop=True
                )
                zr_sb = blk_pool.tile([BLK, h2], F32, tag="zr")
                nc.scalar.activation(
                    zr_sb[:], zr_ps[:], mybir.ActivationFunctionType.Sigmoid
                )
                nc.vector.tensor_copy(
                    z_store[:, b * hid : (b + 1) * hid], zr_sb[:, 0:hid]
                )
                nc.sync.dma_start(
                    out=st_store[:R, b * hid : b * hid + hid],
                    in_=state_s[b * BLK : b * BLK + R, :],
                )
                rs = blk_pool.tile([BLK, hid], F32, tag="rs")
                nc.vector.tensor_tensor(
                    out=rs[:],
                    in0=zr_sb[:, hid:h2],
                    in1=st_store[:, b * hid : (b + 1) * hid],
                    op=mybir.AluOpType.mult,
                )
                featb = blk_pool.tile([BLK, hid], F32, tag="featb")
                nc.vector.memset(featb[:], 0.0)
                nc.sync.dma_start(
                    out=featb[:R, :], in_=feat_s[b * BLK : b * BLK + R, :]
                )
                tp_f = mm_psum.tile([hid, BLK], F32, tag="mm")
                nc.tensor.transpose(out=tp_f[:], in_=featb[:], identity=identity[:])
                featT = blk_pool.tile([hid, BLK], F32, tag="featT")
                nc.vector.tensor_copy(featT[:], tp_f[:])
                tp_r = mm_psum.tile([hid, BLK], F32, tag="mm")
                nc.tensor.transpose(out=tp_r[:], in_=rs[:], identity=identity[:])
                rsT = blk_pool.tile([hid, BLK], F32, tag="rsT")
                nc.vector.tensor_copy(rsT[:], tp_r[:])
                y2_ps = mm_psum.tile([BLK, hid], F32, tag="mm")
                nc.tensor.matmul(
                    y2_ps[:], lhsT=featT[:], rhs=wctop_sb[:], start=True, stop=False
                )
                nc.tensor.matmul(
                    y2_ps[:], lhsT=rsT[:], rhs=wcbot_sb[:], start=False, stop=True
                )
                y2_sb = blk_pool.tile([BLK, hid], BF16, tag="y2")
                nc.vector.tensor_copy(y2_sb[:], y2_ps[:])
                nc.sync.dma_start(
                    out=y2_shard[b * BLK : b * BLK + R, :], in_=y2_sb[:R, :]
                )

            for g0, g1, c in chunks:
                kg = g1 - g0
                nidx = kg * BLK
                tbl = x1b[0:split, :] if c == 0 else x1b[split:n_nodes, :]
                msgs = msg_pool.tile([BLK, MAX_G_CHUNK * h2], BF16, tag="m1")
                out_ap = msgs[:, : kg * h2].rearrange("p (t w) -> p t w", w=h2)
                nc.gpsimd.dma_gather(
                    out_ap,
                    tbl,
                    idx16_sb[:, 8 * g0 : 8 * g1],
                    nidx,
                    nidx,
                    h2,
                    queue_num=next_q(),
                    single_packet=False,
                )
                for g in range(g0, g1):
                    b = blk_of_g[g]
                    if b not in psum_of:
                        psum_of[b] = agg_psum.tile([h2, BLK], F32, tag="agg", name=f"agga{b}")
                    oh = oh_pool.tile([BLK, BLK], BF16, tag="oh")
                    nc.vector.tensor_scalar(
                        out=oh[:],
                        in0=iota_h[:],
                        scalar1=dst_sb[:, g : g + 1],
                        scalar2=w_sb[:, g : g + 1],
                        op0=mybir.AluOpType.is_equal,
                        op1=mybir.AluOpType.mult,
                    )
                    gl = (g - g0) * h2
                    nc.tensor.matmul(
                        out=psum_of[b][:],
                        lhsT=msgs[:, gl : gl + h2],
                        rhs=oh[:],
                        start=(g == first_g[b]),
                        stop=(g == last_g[b]),
                    )
                    if g == last_g[b]:
                        tail_a(b)

            # ================= Phase B: AllGather y2 ========================
            nc.gpsimd.collective_compute(
                "AllGather",
                mybir.AluOpType.bypass,
                replica_groups=[list(range(n_cores))],
                ins=[y2_shard[:, :]],
                outs=[y2_full[:, :]],
            )

            # ============== Phase C: pass-2 aggregation + output ===========
            def tail_c(b):
                R = rows_of(b)
                psum_c = psum_of.pop(b)
                nc.tensor.matmul(
                    psum_c[:, :hid], lhsT=ones1[:], rhs=bc_sb[:], start=False, stop=True
                )
                c_sb = blk_pool.tile([BLK, hid], F32, tag="c")
                nc.scalar.activation(
                    c_sb[:], psum_c[:, :hid], mybir.ActivationFunctionType.Tanh
                )
                # new_state = c + z*(state - c)
                t1 = blk_pool.tile([BLK, hid], F32, tag="t1")
                nc.vector.tensor_tensor(
                    out=t1[:],
                    in0=st_store[:, b * hid : (b + 1) * hid],
                    in1=c_sb[:],
                    op=mybir.AluOpType.subtract,
                )
                t2 = blk_pool.tile([BLK, hid], F32, tag="t2")
                nc.vector.tensor_tensor(
                    out=t2[:],
                    in0=t1[:],
                    in1=z_store[:, b * hid : (b + 1) * hid],
                    op=mybir.AluOpType.mult,
                )
                ns = blk_pool.tile([BLK, hid], F32, tag="ns")
                nc.vector.tensor_tensor(
                    out=ns[:], in0=t2[:], in1=c_sb[:], op=mybir.AluOpType.add
                )
                nc.sync.dma_start(
                    out=out[b * BLK : b * BLK + R, :], in_=ns[:R, :]
                )

            y2_pairs = y2_full[:, :].rearrange("(n two) h -> n (two h)", two=2)
            for g0, g1, c in chunks:
                kg = g1 - g0
                nidx = kg * BLK
                msgs2 = msg_pool.tile([BLK, MAX_G_CHUNK * h2], BF16, tag="m2")
                out_ap = msgs2[:, : kg * h2].rearrange("p (t w) -> p t w", w=h2)
                nc.gpsimd.dma_gather(
                    out_ap,
                    y2_pairs,
                    idx16p_sb[:, 8 * g0 : 8 * g1],
                    nidx,
                    nidx,
                    h2,
                    queue_num=next_q(),
                    single_packet=False,
                )
                for g in range(g0, g1):
                    b = blk_of_g[g]
                    if b not in psum_of:
                        psum_of[b] = agg_psum.tile([BLK, BLK], F32, tag="agg", name=f"aggc{b}")
                    gl = (g - g0) * h2
                    ohp = oh_pool.tile([BLK, 2 * BLK], BF16, tag="ohf")
                    nc.vector.tensor_scalar(
                        out=ohp[:],
                        in0=iota2_h[:],
                        scalar1=dstp_sb[:, g : g + 1],
                        scalar2=w_sb[:, g : g + 1],
                        op0=mybir.AluOpType.is_equal,
                        op1=mybir.AluOpType.mult,
                    )
                    nc.tensor.matmul(
                        out=psum_of[b][:, :hid],
                        lhsT=ohp[:, 0:BLK],
                        rhs=msgs2[:, gl : gl + hid],
                        start=(g == first_g[b]),
                        stop=False,
                    )
                    nc.tensor.matmul(
                        out=psum_of[b][:, :hid],
                        lhsT=ohp[:, BLK : 2 * BLK],
                        rhs=msgs2[:, gl + hid : gl + h2],
                        start=False,
                        stop=False,
                    )
                    if g == last_g[b]:
                        tail_c(b)

    nc.finalize()
    return nc


def run(feat, state, src, dst, edge_weight, Wzr, bzr, Wc, bc, trace=False):
    """Build + run on 8 cores; returns (new_state, BassKernelResults)."""
    n_nodes, hid = feat.shape
    n_cores = N_CORES
    shard = n_nodes // n_cores

    idx16, idx16p, dst_t, dstp_t, w_t, plan = _prep_edges(
        dst, src, edge_weight, n_nodes, n_cores
    )
    import ml_dtypes

    pos = plan["pos"]
    # global permutation: node (p, l) lives at row p*shard + pos[p, l]
    inv = np.empty((n_cores, shard), np.int64)
    for p in range(n_cores):
        inv[p, pos[p]] = np.arange(shard)
    x1 = np.concatenate([feat, state], axis=1)
    x1p = np.empty_like(x1)
    for p in range(n_cores):
        x1p[p * shard : (p + 1) * shard] = x1[p * shard : (p + 1) * shard][inv[p]]
    x1b = np.ascontiguousarray(x1p.astype(ml_dtypes.bfloat16))

    nc = _build(n_nodes, hid, plan, n_cores)

    in_maps = []
    for p in range(n_cores):
        in_maps.append(
            {
                "x1b": x1b,
                "feat_s": np.ascontiguousarray(
                    feat[p * shard : (p + 1) * shard][inv[p]]
                ),
                "state_s": np.ascontiguousarray(
                    state[p * shard : (p + 1) * shard][inv[p]]
                ),
                "idx16": np.ascontiguousarray(idx16[p]),
                "idx16p": np.ascontiguousarray(idx16p[p]),
                "dst_t": np.ascontiguousarray(dst_t[p]),
                "dstp_t": np.ascontiguousarray(dstp_t[p]),
                "w_t": np.ascontiguousarray(w_t[p]),
                "wzr": np.ascontiguousarray(Wzr, dtype=np.float32),
                "bzr": np.ascontiguousarray(bzr.reshape(1, -1), dtype=np.float32),
                "wc": np.ascontiguousarray(Wc, dtype=np.float32),
                "bc": np.ascontiguousarray(bc.reshape(1, -1), dtype=np.float32),
            }
        )

    res = run_bass_kernel_spmd(
        nc, in_maps, core_ids=list(range(n_cores)), trace=trace
    )
    shards = [res.results[p]["out"][pos[p]] for p in range(n_cores)]
    return np.concatenate(shards, axis=0), res


def kernel(feat, state, src, dst, edge_weight, Wzr, bzr, Wc, bc):
    out, _ = run(feat, state, src, dst, edge_weight, Wzr, bzr, Wc, bc, trace=False)
    return out



# revision 21
# speedup vs baseline: 1.0656x; 1.0656x over previous
"""DCGRU cell on 8 Trainium2 NeuronCores.

Strategy (dst-sharded graph partitioning, overlapped AllGather):
  - Nodes are sharded into 8 contiguous ranges (one per core). Edges are
    assigned to the core owning dst; within a core, dst nodes are processed
    in blocks of 128 grouped into super-blocks of SB_BLOCKS for gathering.
  - Pass 1 gathers x1 = [feat, state] rows (bf16, 256B) per edge and
    segment-sums them via one-hot matmuls into PSUM (transposed layout).
    zr/r^T/y2 are produced without PE transposes: the host supplies feat^T
    and state^T so y2 = feat @ Wc_top + (r*state) @ Wc_bot uses featT/rsT
    directly as lhsT.
  - y2 (bf16) is AllGathered in KC block-range chunks, each issued as soon
    as its blocks finish in pass 1, so the collective overlaps phase-A
    compute (collective cost model: 15us fixed + bytes/40GBps).
  - Pass 2 gathers y2 rows (128B) directly, sweeping source-chunks k-outer
    so sweep k only waits on collective chunk k; per (block, sweep) partial
    PSUM aggregates accumulate into an SBUF accumulator.
"""

import numpy as np

import concourse.bass as bass
import concourse.bacc as bacc
import concourse.mybir as mybir
import concourse.tile as tile
from concourse.bass_utils import run_bass_kernel_spmd
from concourse.library_config import mlp
from concourse.masks import make_identity

N_NODES = 50000
N_EDGES = 640000
HID = 64
N_CORES = 8
BLK = 128           # dst nodes per block (= PSUM partition dim)
SB_BLOCKS = 4       # dst blocks per super-block (PSUM live tiles = SB_BLOCKS)
MAX_G_CHUNK = 32    # cap on groups per dma_gather instruction
KC = 3              # source chunks for the AllGather / pass-2 sweeps

F32 = mybir.dt.float32
BF16 = mybir.dt.bfloat16
I16 = mybir.dt.int16


def _kchunk_blocks(nblk):
    """Block ranges per source chunk (roughly even)."""
    base = nblk // KC
    rem = nblk % KC
    sizes = [base + (1 if i < rem else 0) for i in range(KC)]
    bounds = np.concatenate([[0], np.cumsum(sizes)])
    return bounds  # len KC+1, bounds[-1] == nblk


def _build_tables(e_owner, e_ukey, e_idxval, e_dloc, e_w, unit_off, n_units):
    """Slot edges into (group, lane) and build idx16/dst/w tables.

    e_* are per-edge arrays; e_ukey is the unit id (dense, 0..n_units-1);
    e_idxval is the int16 gather index value. Returns (idx16, dst_t, w_t).
    """
    e = len(e_owner)
    ngroups = int(unit_off[-1])
    ck = e_owner * n_units + e_ukey
    order = np.argsort(ck, kind="stable")
    ck_s = ck[order]
    owner_s = e_owner[order]
    idxval_s = e_idxval[order]
    ukey_s = e_ukey[order]
    dloc_s = e_dloc[order]
    w_s = e_w[order]
    bucket_start = np.searchsorted(ck_s, np.arange(N_CORES * n_units))
    rank = np.arange(e) - bucket_start[ck_s]
    g_global = unit_off[ukey_s] + rank // BLK
    lane = rank % BLK

    idx16 = np.zeros((N_CORES, 16, 8 * ngroups), np.int16)
    dst_t = np.zeros((N_CORES, BLK, ngroups), np.float32)
    w_t = np.zeros((N_CORES, BLK, ngroups), np.float32)
    idx16[owner_s, lane % 16, 8 * g_global + lane // 16] = idxval_s.astype(np.int16)
    dst_t[owner_s, lane, g_global] = dloc_s
    w_t[owner_s, lane, g_global] = w_s
    idx16 = np.tile(idx16, (1, 8, 1))
    return idx16, dst_t, w_t


def _prep_edges(dst, src, edge_weight, n_nodes, n_cores):
    """Partition edges by dst core/block; build pass-1 and pass-2 tables."""
    shard = n_nodes // n_cores
    nblk = (shard + BLK - 1) // BLK
    split = n_nodes // 2
    e = len(dst)

    dsts = dst.astype(np.int64)
    srcs = src.astype(np.int64)
    owner = dsts // shard
    local = dsts - owner * shard

    # Balance in-degree across blocks: per core, deal nodes (sorted by
    # in-degree, desc) round-robin over blocks. pos[core, orig_local] is the
    # node's new row; node tables / shard rows / outputs use this order.
    deg = np.zeros(n_nodes, np.int64)
    np.add.at(deg, dsts, 1)
    pos = np.empty((n_cores, shard), np.int64)
    blk_fill = np.empty(nblk, np.int64)
    cap = np.full(nblk, BLK, np.int64)
    cap[nblk - 1] = shard - (nblk - 1) * BLK
    for p in range(n_cores):
        nodes = np.argsort(-deg[p * shard : (p + 1) * shard], kind="stable")
        blk_fill[:] = 0
        bi = 0
        for n in nodes:
            while blk_fill[bi % nblk] >= cap[bi % nblk]:
                bi += 1
            b = bi % nblk
            pos[p, n] = b * BLK + blk_fill[b]
            blk_fill[b] += 1
            bi += 1
    newloc = pos[owner, local]
    b_of = newloc // BLK
    dloc = (newloc % BLK).astype(np.float32)
    w_f = edge_weight.astype(np.float32)

    # permuted global src position
    src_pos = pos[srcs // shard, srcs % shard] + (srcs // shard) * shard

    # ---------------- pass 1: units = (super-block, class, block) ----------
    cls = (src_pos >= split).astype(np.int64)
    src_local1 = src_pos - cls * split

    cnt = np.zeros((n_cores, nblk, 2), np.int64)
    np.add.at(cnt, (owner, b_of, cls), 1)
    gpbc = -(-cnt.max(axis=0) // BLK)  # [nblk, 2]
    empty = gpbc.sum(axis=1) == 0
    gpbc[empty, 0] = 1

    unit_order = []
    for sb0 in range(0, nblk, SB_BLOCKS):
        sbb = range(sb0, min(sb0 + SB_BLOCKS, nblk))
        for c in range(2):
            for b in sbb:
                unit_order.append((b, c))
    unit_sizes = np.array([gpbc[b, c] for b, c in unit_order], np.int64)
    unit_off = np.concatenate([[0], np.cumsum(unit_sizes)])
    ngroups = int(unit_off[-1])
    unit_idx = {bc: i for i, bc in enumerate(unit_order)}

    blk_of_g = np.zeros(ngroups, np.int64)
    for i, (b, c) in enumerate(unit_order):
        blk_of_g[unit_off[i] : unit_off[i + 1]] = b
    first_g = np.full(nblk, -1, np.int64)
    last_g = np.full(nblk, -1, np.int64)
    for g in range(ngroups):
        b = blk_of_g[g]
        if first_g[b] < 0:
            first_g[b] = g
        last_g[b] = g

    # gather chunks: contiguous same-class unit runs within a super-block
    chunks = []
    i = 0
    while i < len(unit_order):
        c = unit_order[i][1]
        sb = unit_order[i][0] // SB_BLOCKS
        j = i
        while (
            j < len(unit_order)
            and unit_order[j][1] == c
            and unit_order[j][0] // SB_BLOCKS == sb
        ):
            j += 1
        g0, g1 = int(unit_off[i]), int(unit_off[j])
        for s in range(g0, g1, MAX_G_CHUNK):
            if s < g1:
                chunks.append((s, min(s + MAX_G_CHUNK, g1), c))
        i = j

    ukey1 = np.array([unit_idx[(b, c)] for b, c in zip(b_of, cls)], np.int64)
    idx16, dst_t, w_t = _build_tables(
        owner, ukey1, src_local1, dloc, w_f, unit_off, len(unit_order)
    )

    # ---------------- pass 2: units = (kchunk, block) ----------------------
    kb = _kchunk_blocks(nblk)  # block bounds, len KC+1
    krow = kb * BLK  # row bounds within shard (last may exceed shard)
    krow[-1] = shard
    rows_k = np.diff(krow)  # rows per chunk per core

    assert all(r % 2 == 0 for r in rows_k), "pair trick needs even chunk rows"
    sp_core = src_pos // shard
    sp_local = src_pos - sp_core * shard
    sp_blk = sp_local // BLK
    kc_of = np.searchsorted(kb[1:], sp_blk, side="right")
    crow = sp_core * rows_k[kc_of] + (sp_local - krow[kc_of])  # row in y2f[k]
    idxval2 = crow // 2
    par2 = (crow % 2).astype(np.float32)

    cnt2 = np.zeros((n_cores, KC, nblk), np.int64)
    np.add.at(cnt2, (owner, kc_of, b_of), 1)
    gp2 = -(-cnt2.max(axis=0) // BLK)  # [KC, nblk]
    gp2 = np.maximum(gp2, 1)

    unit_order2 = [(k, b) for k in range(KC) for b in range(nblk)]
    unit_sizes2 = np.array([gp2[k, b] for k, b in unit_order2], np.int64)
    unit_off2 = np.concatenate([[0], np.cumsum(unit_sizes2)])
    ngroups2 = int(unit_off2[-1])

    blk_of_g2 = np.zeros(ngroups2, np.int64)
    for i, (k, b) in enumerate(unit_order2):
        blk_of_g2[unit_off2[i] : unit_off2[i + 1]] = b
    # first/last group per (block, k)
    fc2 = np.zeros((nblk, KC), np.int64)
    lc2 = np.zeros((nblk, KC), np.int64)
    for i, (k, b) in enumerate(unit_order2):
        fc2[b, k] = unit_off2[i]
        lc2[b, k] = unit_off2[i + 1] - 1

    chunks2 = []
    for k in range(KC):
        g0 = int(unit_off2[k * nblk])
        g1 = int(unit_off2[(k + 1) * nblk])
        for s in range(g0, g1, MAX_G_CHUNK):
            chunks2.append((s, min(s + MAX_G_CHUNK, g1), k))

    ukey2 = kc_of * nblk + b_of
    idx16c, dst2_t, w2_t = _build_tables(
        owner, ukey2, idxval2, dloc + BLK * par2, w_f, unit_off2, KC * nblk
    )

    plan = {
        "chunks": chunks,
        "chunks2": chunks2,
        "blk_of_g": [int(x) for x in blk_of_g],
        "first_g": [int(x) for x in first_g],
        "last_g": [int(x) for x in last_g],
        "blk_of_g2": [int(x) for x in blk_of_g2],
        "fc2": fc2,
        "lc2": lc2,
        "kb": [int(x) for x in kb],
        "rows_k": [int(x) for x in rows_k],
        "nblk": nblk,
        "ngroups": ngroups,
        "ngroups2": ngroups2,
        "pos": pos,
    }
    return idx16, idx16c, dst_t, dst2_t, w_t, w2_t, plan


def _build(n_nodes, hid, plan, n_cores, n_queues=4):
    """Build the SPMD Bass program from the edge plan."""
    shard = n_nodes // n_cores
    nblk = plan["nblk"]
    ngroups = plan["ngroups"]
    ngroups2 = plan["ngroups2"]
    chunks = plan["chunks"]
    chunks2 = plan["chunks2"]
    blk_of_g = plan["blk_of_g"]
    first_g = plan["first_g"]
    last_g = plan["last_g"]
    blk_of_g2 = plan["blk_of_g2"]
    fc2 = plan["fc2"]
    lc2 = plan["lc2"]
    kb = plan["kb"]
    rows_k = plan["rows_k"]
    split = n_nodes // 2
    h2 = 2 * hid

    nc = bacc.Bacc(
        None,
        num_devices=n_cores,
        num_swdge_queues=n_queues,
        dynamic_dma_scratch_size=16 * BLK * MAX_G_CHUNK,
    )

    x1b = nc.dram_tensor("x1b", [n_nodes, h2], BF16, kind="ExternalInput")
    state_s = nc.dram_tensor("state_s", [shard, hid], F32, kind="ExternalInput")
    featT_s = nc.dram_tensor("featT_s", [hid, shard], BF16, kind="ExternalInput")
    stateT_s = nc.dram_tensor("stateT_s", [hid, shard], BF16, kind="ExternalInput")
    idx16_d = nc.dram_tensor("idx16", [BLK, 8 * ngroups], I16, kind="ExternalInput")
    idx2_d = nc.dram_tensor("idx2", [BLK, 8 * ngroups2], I16, kind="ExternalInput")
    dst_d = nc.dram_tensor("dst_t", [BLK, ngroups], F32, kind="ExternalInput")
    dst2_d = nc.dram_tensor("dst2_t", [BLK, ngroups2], F32, kind="ExternalInput")
    w_d = nc.dram_tensor("w_t", [BLK, ngroups], F32, kind="ExternalInput")
    w2_d = nc.dram_tensor("w2_t", [BLK, ngroups2], F32, kind="ExternalInput")
    wzr = nc.dram_tensor("wzr", [h2, h2], F32, kind="ExternalInput")
    bzr = nc.dram_tensor("bzr", [1, h2], F32, kind="ExternalInput")
    wc = nc.dram_tensor("wc", [h2, hid], F32, kind="ExternalInput")
    bc = nc.dram_tensor("bc", [1, hid], F32, kind="ExternalInput")
    out = nc.dram_tensor("out", [shard, hid], F32, kind="ExternalOutput")

    y2s = [
        nc.dram_tensor(f"y2s{k}", [rows_k[k], hid], BF16, kind="Internal")
        for k in range(KC)
    ]
    y2f = [
        nc.dram_tensor(
            f"y2f{k}", [n_cores * rows_k[k], hid], BF16, kind="Internal",
            addr_space="Shared",
        )
        for k in range(KC)
    ]

    mx1 = max(g1 - g0 for g0, g1, _ in chunks)
    mx2 = max(g1 - g0 for g0, g1, _ in chunks2)
    qn = [0]

    def next_q():
        q = qn[0]
        qn[0] = (qn[0] + 1) % n_queues
        return q

    def rows_of(b):
        return BLK if b < nblk - 1 else shard - (nblk - 1) * BLK

    with tile.TileContext(nc) as tc:
        with (
            tc.tile_pool(name="const", bufs=1) as const_pool,
            tc.tile_pool(name="store", bufs=1) as store_pool,
            tc.tile_pool(name="msg", bufs=2) as msg_pool,
            tc.tile_pool(name="oh", bufs=6) as oh_pool,
            tc.tile_pool(name="blk", bufs=6) as blk_pool,
            tc.tile_pool(name="agg_ps", bufs=SB_BLOCKS, space="PSUM") as agg_psum,
            tc.tile_pool(name="mm_ps", bufs=2, space="PSUM") as mm_psum,
        ):
            nc.gpsimd.load_library(mlp)
            # ---- constants ----
            iota_i = const_pool.tile([BLK, BLK], mybir.dt.int32)
            nc.gpsimd.iota(iota_i[:], pattern=[[1, BLK]], base=0, channel_multiplier=0)
            iota_h = const_pool.tile([BLK, BLK], BF16)
            nc.vector.tensor_copy(iota_h[:], iota_i[:])
            iota2_i = const_pool.tile([BLK, 2 * BLK], mybir.dt.int32)
            nc.gpsimd.iota(
                iota2_i[:], pattern=[[1, 2 * BLK]], base=0, channel_multiplier=0
            )
            iota2_h = const_pool.tile([BLK, 2 * BLK], BF16)
            nc.vector.tensor_copy(iota2_h[:], iota2_i[:])
            ones1 = const_pool.tile([1, BLK], F32)
            nc.vector.memset(ones1[:], 1.0)
            wzr_sb = const_pool.tile([h2, h2], F32)
            nc.sync.dma_start(out=wzr_sb[:], in_=wzr[:, :])
            bzr_sb = const_pool.tile([1, h2], F32)
            nc.sync.dma_start(out=bzr_sb[:], in_=bzr[:, :])
            wct_f32 = const_pool.tile([hid, hid], F32)
            nc.sync.dma_start(out=wct_f32[:], in_=wc[0:hid, :])
            wcb_f32 = const_pool.tile([hid, hid], F32)
            nc.sync.dma_start(out=wcb_f32[:], in_=wc[hid:h2, :])
            wctop_sb = const_pool.tile([hid, hid], BF16)
            nc.vector.tensor_copy(wctop_sb[:], wct_f32[:])
            wcbot_sb = const_pool.tile([hid, hid], BF16)
            nc.vector.tensor_copy(wcbot_sb[:], wcb_f32[:])
            bc_sb = const_pool.tile([1, hid], F32)
            nc.sync.dma_start(out=bc_sb[:], in_=bc[:, :])

            # ---- persistent stores ----
            idx16_sb = store_pool.tile([BLK, 8 * ngroups], I16)
            nc.sync.dma_start(out=idx16_sb[:], in_=idx16_d[:, :])
            idx2_sb = store_pool.tile([BLK, 8 * ngroups2], I16)
            nc.sync.dma_start(out=idx2_sb[:], in_=idx2_d[:, :])
            dst_sb = store_pool.tile([BLK, ngroups], F32)
            nc.sync.dma_start(out=dst_sb[:], in_=dst_d[:, :])
            w_sb = store_pool.tile([BLK, ngroups], F32)
            nc.sync.dma_start(out=w_sb[:], in_=w_d[:, :])
            dst2_sb = store_pool.tile([BLK, ngroups2], F32)
            nc.sync.dma_start(out=dst2_sb[:], in_=dst2_d[:, :])
            w2_sb = store_pool.tile([BLK, ngroups2], F32)
            nc.sync.dma_start(out=w2_sb[:], in_=w2_d[:, :])

            nfull = (nblk - 1) * BLK  # rows in full blocks
            featT_store = store_pool.tile([hid, nblk * BLK], BF16)
            nc.vector.memset(featT_store[:], 0.0)
            nc.sync.dma_start(out=featT_store[:, 0:shard], in_=featT_s[:, :])
            stateT_store = store_pool.tile([hid, nblk * BLK], BF16)
            nc.vector.memset(stateT_store[:], 0.0)
            nc.sync.dma_start(out=stateT_store[:, 0:shard], in_=stateT_s[:, :])
            st_store = store_pool.tile([BLK, nblk * hid], F32)
            nc.vector.memset(st_store[:], 0.0)
            nc.sync.dma_start(
                out=st_store[:, 0 : (nblk - 1) * hid].rearrange(
                    "p (b h) -> p b h", h=hid
                ),
                in_=state_s[0:nfull, :].rearrange("(b p) h -> p b h", p=BLK),
            )
            nc.sync.dma_start(
                out=st_store[: shard - nfull, (nblk - 1) * hid : nblk * hid],
                in_=state_s[nfull:shard, :],
            )
            z_store = store_pool.tile([BLK, nblk * hid], F32)
            acc_store = store_pool.tile([BLK, nblk * hid], F32)

            # ============== Phase A: pass-1 aggregation + y2 ===============
            psum_of = {}
            done_blocks = [0]
            coll_emitted = [0]

            def tail_a(b):
                """Post-aggregation per-block work for pass 1."""
                R = rows_of(b)
                k = int(np.searchsorted(kb[1:], b, side="right"))
                aggT_ps = psum_of.pop(b)
                aggT = blk_pool.tile([h2, BLK], F32, tag="aggT")
                nc.vector.tensor_copy(aggT[:], aggT_ps[:])
                zr_ps = mm_psum.tile([BLK, hid], F32, tag="mm")
                nc.tensor.matmul(
                    zr_ps[:], lhsT=aggT[:], rhs=wzr_sb[:, 0:hid], start=True, stop=False
                )
                nc.tensor.matmul(
                    zr_ps[:], lhsT=ones1[:], rhs=bzr_sb[:, 0:hid], start=False, stop=True
                )
                nc.scalar.activation(
                    z_store[:, b * hid : (b + 1) * hid],
                    zr_ps[:],
                    mybir.ActivationFunctionType.Sigmoid,
                )
                rT_ps = mm_psum.tile([hid, BLK], F32, tag="mm")
                nc.tensor.matmul(
                    rT_ps[:], lhsT=wzr_sb[:, hid:h2], rhs=aggT[:], start=True, stop=False
                )
                nc.tensor.matmul(
                    rT_ps[:], lhsT=bzr_sb[:, hid:h2], rhs=ones1[:], start=False, stop=True
                )
                rT_sb = blk_pool.tile([hid, BLK], BF16, tag="rT")
                nc.scalar.activation(
                    rT_sb[:], rT_ps[:], mybir.ActivationFunctionType.Sigmoid
                )
                rsT = blk_pool.tile([hid, BLK], BF16, tag="rsT")
                nc.vector.tensor_tensor(
                    out=rsT[:],
                    in0=rT_sb[:],
                    in1=stateT_store[:, b * BLK : (b + 1) * BLK],
                    op=mybir.AluOpType.mult,
                )
                y2_ps = mm_psum.tile([BLK, hid], F32, tag="mm")
                nc.tensor.matmul(
                    y2_ps[:],
                    lhsT=featT_store[:, b * BLK : (b + 1) * BLK],
                    rhs=wctop_sb[:],
                    start=True,
                    stop=False,
                )
                nc.tensor.matmul(
                    y2_ps[:], lhsT=rsT[:], rhs=wcbot_sb[:], start=False, stop=True
                )
                y2_sb = blk_pool.tile([BLK, hid], BF16, tag="y2")
                nc.vector.tensor_copy(y2_sb[:], y2_ps[:])
                r0 = b * BLK - kb[k] * BLK  # row offset within source chunk k
                nc.sync.dma_start(out=y2s[k][r0 : r0 + R, :], in_=y2_sb[:R, :])
                done_blocks[0] += 1

            def maybe_emit_colls():
                while coll_emitted[0] < KC and done_blocks[0] >= kb[coll_emitted[0] + 1]:
                    k = coll_emitted[0]
                    nc.gpsimd.collective_compute(
                        "AllGather",
                        mybir.AluOpType.bypass,
                        replica_groups=[list(range(n_cores))],
                        ins=[y2s[k][:, :]],
                        outs=[y2f[k][:, :]],
                    )
                    coll_emitted[0] += 1

            for g0, g1, c in chunks:
                kg = g1 - g0
                nidx = kg * BLK
                tbl = x1b[0:split, :] if c == 0 else x1b[split:n_nodes, :]
                msgs = msg_pool.tile([BLK, mx1 * h2], BF16, tag="m1")
                out_ap = msgs[:, : kg * h2].rearrange("p (t w) -> p t w", w=h2)
                nc.gpsimd.dma_gather(
                    out_ap,
                    tbl,
                    idx16_sb[:, 8 * g0 : 8 * g1],
                    nidx,
                    nidx,
                    h2,
                    queue_num=next_q(),
                    single_packet=False,
                )
                for g in range(g0, g1):
                    b = blk_of_g[g]
                    if b not in psum_of:
                        psum_of[b] = agg_psum.tile(
                            [h2, BLK], F32, tag="agg", name=f"agga{b}"
                        )
                    oh = oh_pool.tile([BLK, BLK], BF16, tag="oh")
                    nc.vector.tensor_scalar(
                        out=oh[:],
                        in0=iota_h[:],
                        scalar1=dst_sb[:, g : g + 1],
                        scalar2=w_sb[:, g : g + 1],
                        op0=mybir.AluOpType.is_equal,
                        op1=mybir.AluOpType.mult,
                    )
                    gl = (g - g0) * h2
                    nc.tensor.matmul(
                        out=psum_of[b][:],
                        lhsT=msgs[:, gl : gl + h2],
                        rhs=oh[:],
                        start=(g == first_g[b]),
                        stop=(g == last_g[b]),
                    )
                    if g == last_g[b]:
                        tail_a(b)
                maybe_emit_colls()

            # ============== Phase C: pass-2 sweeps over source chunks =======
            def acc_c(b, k, psum_c):
                """Fold sweep-k partial aggregate for block b into SBUF/output."""
                R = rows_of(b)
                sl = slice(b * hid, (b + 1) * hid)
                if k == 0:
                    nc.vector.tensor_copy(acc_store[:, sl], psum_c[:])
                    return
                if k < KC - 1:
                    nc.vector.tensor_tensor(
                        out=acc_store[:, sl],
                        in0=psum_c[:],
                        in1=acc_store[:, sl],
                        op=mybir.AluOpType.add,
                    )
                    return
                t0 = blk_pool.tile([BLK, hid], F32, tag="t0")
                nc.vector.tensor_tensor(
                    out=t0[:], in0=psum_c[:], in1=acc_store[:, sl],
                    op=mybir.AluOpType.add,
                )
                c_sb = blk_pool.tile([BLK, hid], F32, tag="c")
                nc.scalar.activation(
                    c_sb[:], t0[:], mybir.ActivationFunctionType.Tanh
                )
                # new_state = c + z*(state - c)
                t1 = blk_pool.tile([BLK, hid], F32, tag="t1")
                nc.vector.tensor_tensor(
                    out=t1[:],
                    in0=st_store[:, sl],
                    in1=c_sb[:],
                    op=mybir.AluOpType.subtract,
                )
                t2 = blk_pool.tile([BLK, hid], F32, tag="t2")
                nc.vector.tensor_tensor(
                    out=t2[:],
                    in0=t1[:],
                    in1=z_store[:, sl],
                    op=mybir.AluOpType.mult,
                )
                ns = blk_pool.tile([BLK, hid], F32, tag="ns")
                nc.vector.tensor_tensor(
                    out=ns[:], in0=t2[:], in1=c_sb[:], op=mybir.AluOpType.add
                )
                nc.sync.dma_start(out=out[b * BLK : b * BLK + R, :], in_=ns[:R, :])

            psum_c_of = {}
            for g0, g1, k in chunks2:
                kg = g1 - g0
                nidx = kg * BLK
                tbl = y2f[k][:, :].rearrange("(n two) h -> n (two h)", two=2)
                msgs2 = msg_pool.tile([BLK, mx2 * h2], BF16, tag="m2")
                out_ap = msgs2[:, : kg * h2].rearrange("p (t w) -> p t w", w=h2)
                nc.gpsimd.dma_gather(
                    out_ap,
                    tbl,
                    idx2_sb[:, 8 * g0 : 8 * g1],
                    nidx,
                    nidx,
                    h2,
                    queue_num=next_q(),
                    single_packet=False,
                )
                for g in range(g0, g1):
                    b = blk_of_g2[g]
                    if b not in psum_c_of:
                        psum_c_of[b] = agg_psum.tile(
                            [BLK, hid], F32, tag="agg", name=f"aggc{b}k{k}"
                        )
                    ohp = oh_pool.tile([BLK, 2 * BLK], BF16, tag="ohp")
                    nc.vector.tensor_scalar(
                        out=ohp[:],
                        in0=iota2_h[:],
                        scalar1=dst2_sb[:, g : g + 1],
                        scalar2=w2_sb[:, g : g + 1],
                        op0=mybir.AluOpType.is_equal,
                        op1=mybir.AluOpType.mult,
                    )
                    gl = (g - g0) * h2
                    last_in_sweep = g == lc2[b][k]
                    final = k == KC - 1
                    nc.tensor.matmul(
                        out=psum_c_of[b][:],
                        lhsT=ohp[:, 0:BLK],
                        rhs=msgs2[:, gl : gl + hid],
                        start=(g == fc2[b][k]),
                        stop=False,
                    )
                    nc.tensor.matmul(
                        out=psum_c_of[b][:],
                        lhsT=ohp[:, BLK : 2 * BLK],
                        rhs=msgs2[:, gl + hid : gl + h2],
                        start=False,
                        stop=(last_in_sweep and not final),
                    )
                    if last_in_sweep:
                        psum_c = psum_c_of.pop(b)
                        if final:
                            nc.tensor.matmul(
                                psum_c[:], lhsT=ones1[:], rhs=bc_sb[:],
                                start=False, stop=True,
                            )
                        acc_c(b, k, psum_c)

    nc.finalize()
    return nc


def run(feat, state, src, dst, edge_weight, Wzr, bzr, Wc, bc, trace=False):
    """Build + run on 8 cores; returns (new_state, BassKernelResults)."""
    n_nodes, hid = feat.shape
    n_cores = N_CORES
    shard = n_nodes // n_cores

    idx16, idx16c, dst_t, dst2_t, w_t, w2_t, plan = _prep_edges(
        dst, src, edge_weight, n_nodes, n_cores
    )
    import ml_dtypes

    pos = plan["pos"]
    # global permutation: node (p, l) lives at row p*shard + pos[p, l]
    inv = np.empty((n_cores, shard), np.int64)
    for p in range(n_cores):
        inv[p, pos[p]] = np.arange(shard)
    x1 = np.concatenate([feat, state], axis=1)
    x1p = np.empty_like(x1)
    for p in range(n_cores):
        x1p[p * shard : (p + 1) * shard] = x1[p * shard : (p + 1) * shard][inv[p]]
    x1b = np.ascontiguousarray(x1p.astype(ml_dtypes.bfloat16))

    nc = _build(n_nodes, hid, plan, n_cores)

    in_maps = []
    for p in range(n_cores):
        feat_p = feat[p * shard : (p + 1) * shard][inv[p]]
        state_p = state[p * shard : (p + 1) * shard][inv[p]]
        in_maps.append(
            {
                "x1b": x1b,
                "state_s": np.ascontiguousarray(state_p),
                "featT_s": np.ascontiguousarray(feat_p.T.astype(ml_dtypes.bfloat16)),
                "stateT_s": np.ascontiguousarray(state_p.T.astype(ml_dtypes.bfloat16)),
                "idx16": np.ascontiguousarray(idx16[p]),
                "idx2": np.ascontiguousarray(idx16c[p]),
                "dst_t": np.ascontiguousarray(dst_t[p]),
                "dst2_t": np.ascontiguousarray(dst2_t[p]),
                "w_t": np.ascontiguousarray(w_t[p]),
                "w2_t": np.ascontiguousarray(w2_t[p]),
                "wzr": np.ascontiguousarray(Wzr, dtype=np.float32),
                "bzr": np.ascontiguousarray(bzr.reshape(1, -1), dtype=np.float32),
                "wc": np.ascontiguousarray(Wc, dtype=np.float32),
                "bc": np.ascontiguousarray(bc.reshape(1, -1), dtype=np.float32),
            }
        )

    res = run_bass_kernel_spmd(
        nc, in_maps, core_ids=list(range(n_cores)), trace=trace
    )
    shards = [res.results[p]["out"][pos[p]] for p in range(n_cores)]
    return np.concatenate(shards, axis=0), res


def kernel(feat, state, src, dst, edge_weight, Wzr, bzr, Wc, bc):
    out, _ = run(feat, state, src, dst, edge_weight, Wzr, bzr, Wc, bc, trace=False)
    return out


# revision 25
# speedup vs baseline: 1.1471x; 1.0765x over previous
"""DCGRU cell on 8 Trainium2 NeuronCores.

Strategy (dst-sharded graph partitioning, overlapped AllGather):
  - Nodes are sharded into 8 contiguous ranges (one per core). Edges are
    assigned to the core owning dst; within a core, dst nodes are processed
    in blocks of 128 grouped into super-blocks of SB_BLOCKS for gathering.
  - Pass 1 gathers x1 = [feat, state] rows (bf16, 256B) per edge and
    segment-sums them via one-hot matmuls into PSUM (transposed layout).
    zr/r^T/y2 are produced without PE transposes: the host supplies feat^T
    and state^T so y2 = feat @ Wc_top + (r*state) @ Wc_bot uses featT/rsT
    directly as lhsT.
  - y2 (bf16) is AllGathered in KC block-range chunks, each issued as soon
    as its blocks finish in pass 1, so the collective overlaps phase-A
    compute (collective cost model: 15us fixed + bytes/40GBps).
  - Pass 2 gathers y2 rows (128B) directly, sweeping source-chunks k-outer
    so sweep k only waits on collective chunk k; per (block, sweep) partial
    PSUM aggregates accumulate into an SBUF accumulator.
"""

import numpy as np

import concourse.bass as bass
import concourse.bacc as bacc
import concourse.mybir as mybir
import concourse.tile as tile
from concourse.bass_utils import run_bass_kernel_spmd
from concourse.library_config import mlp
from concourse.masks import make_identity

N_NODES = 50000
N_EDGES = 640000
HID = 64
N_CORES = 8
BLK = 128           # dst nodes per block (= PSUM partition dim)
SB_BLOCKS = 4       # dst blocks per super-block (PSUM live tiles = SB_BLOCKS)
MAX_G_CHUNK = 32    # cap on groups per dma_gather instruction
KC = 4              # source chunks for the AllGather / pass-2 sweeps

F32 = mybir.dt.float32
BF16 = mybir.dt.bfloat16
I16 = mybir.dt.int16


def _kchunk_blocks(nblk):
    """Block ranges per source chunk: even early chunks, small last chunk
    (the last chunk's collective + sweep sit on the critical path)."""
    last = max(nblk // 7, 1)
    base = nblk - last
    sizes = [base // (KC - 1) + (1 if i < base % (KC - 1) else 0) for i in range(KC - 1)]
    sizes.append(last)
    bounds = np.concatenate([[0], np.cumsum(sizes)])
    return bounds  # len KC+1, bounds[-1] == nblk


def _build_tables(e_owner, e_ukey, e_idxval, e_dloc, e_w, unit_off, n_units):
    """Slot edges into (group, lane) and build idx16/dst/w tables.

    e_* are per-edge arrays; e_ukey is the unit id (dense, 0..n_units-1);
    e_idxval is the int16 gather index value. Returns (idx16, dst_t, w_t).
    """
    e = len(e_owner)
    ngroups = int(unit_off[-1])
    ck = e_owner * n_units + e_ukey
    order = np.argsort(ck, kind="stable")
    ck_s = ck[order]
    owner_s = e_owner[order]
    idxval_s = e_idxval[order]
    ukey_s = e_ukey[order]
    dloc_s = e_dloc[order]
    w_s = e_w[order]
    bucket_start = np.searchsorted(ck_s, np.arange(N_CORES * n_units))
    rank = np.arange(e) - bucket_start[ck_s]
    g_global = unit_off[ukey_s] + rank // BLK
    lane = rank % BLK

    idx16 = np.zeros((N_CORES, 16, 8 * ngroups), np.int16)
    dst_t = np.zeros((N_CORES, BLK, ngroups), np.float32)
    w_t = np.zeros((N_CORES, BLK, ngroups), np.float32)
    idx16[owner_s, lane % 16, 8 * g_global + lane // 16] = idxval_s.astype(np.int16)
    dst_t[owner_s, lane, g_global] = dloc_s
    w_t[owner_s, lane, g_global] = w_s
    idx16 = np.tile(idx16, (1, 8, 1))
    return idx16, dst_t, w_t


def _prep_edges(dst, src, edge_weight, n_nodes, n_cores):
    """Partition edges by dst core/block; build pass-1 and pass-2 tables."""
    shard = n_nodes // n_cores
    nblk = (shard + BLK - 1) // BLK
    split = n_nodes // 2
    e = len(dst)

    dsts = dst.astype(np.int64)
    srcs = src.astype(np.int64)
    owner = dsts // shard
    local = dsts - owner * shard

    # Balance in-degree across blocks: per core, deal nodes (sorted by
    # in-degree, desc) round-robin over blocks. pos[core, orig_local] is the
    # node's new row; node tables / shard rows / outputs use this order.
    deg = np.zeros(n_nodes, np.int64)
    np.add.at(deg, dsts, 1)
    pos = np.empty((n_cores, shard), np.int64)
    blk_fill = np.empty(nblk, np.int64)
    cap = np.full(nblk, BLK, np.int64)
    cap[nblk - 1] = shard - (nblk - 1) * BLK
    for p in range(n_cores):
        nodes = np.argsort(-deg[p * shard : (p + 1) * shard], kind="stable")
        blk_fill[:] = 0
        bi = 0
        for n in nodes:
            while blk_fill[bi % nblk] >= cap[bi % nblk]:
                bi += 1
            b = bi % nblk
            pos[p, n] = b * BLK + blk_fill[b]
            blk_fill[b] += 1
            bi += 1
    newloc = pos[owner, local]
    b_of = newloc // BLK
    dloc = (newloc % BLK).astype(np.float32)
    w_f = edge_weight.astype(np.float32)

    # permuted global src position
    src_pos = pos[srcs // shard, srcs % shard] + (srcs // shard) * shard

    # ---------------- pass 1: units = (super-block, class, block) ----------
    cls = (src_pos >= split).astype(np.int64)
    src_local1 = src_pos - cls * split

    cnt = np.zeros((n_cores, nblk, 2), np.int64)
    np.add.at(cnt, (owner, b_of, cls), 1)
    gpbc = -(-cnt.max(axis=0) // BLK)  # [nblk, 2]
    empty = gpbc.sum(axis=1) == 0
    gpbc[empty, 0] = 1

    unit_order = []
    for sb0 in range(0, nblk, SB_BLOCKS):
        sbb = range(sb0, min(sb0 + SB_BLOCKS, nblk))
        for c in range(2):
            for b in sbb:
                unit_order.append((b, c))
    unit_sizes = np.array([gpbc[b, c] for b, c in unit_order], np.int64)
    unit_off = np.concatenate([[0], np.cumsum(unit_sizes)])
    ngroups = int(unit_off[-1])
    unit_idx = {bc: i for i, bc in enumerate(unit_order)}

    blk_of_g = np.zeros(ngroups, np.int64)
    for i, (b, c) in enumerate(unit_order):
        blk_of_g[unit_off[i] : unit_off[i + 1]] = b
    first_g = np.full(nblk, -1, np.int64)
    last_g = np.full(nblk, -1, np.int64)
    for g in range(ngroups):
        b = blk_of_g[g]
        if first_g[b] < 0:
            first_g[b] = g
        last_g[b] = g

    # gather chunks: contiguous same-class unit runs within a super-block
    chunks = []
    i = 0
    while i < len(unit_order):
        c = unit_order[i][1]
        sb = unit_order[i][0] // SB_BLOCKS
        j = i
        while (
            j < len(unit_order)
            and unit_order[j][1] == c
            and unit_order[j][0] // SB_BLOCKS == sb
        ):
            j += 1
        g0, g1 = int(unit_off[i]), int(unit_off[j])
        for s in range(g0, g1, MAX_G_CHUNK):
            if s < g1:
                chunks.append((s, min(s + MAX_G_CHUNK, g1), c))
        i = j

    ukey1 = np.array([unit_idx[(b, c)] for b, c in zip(b_of, cls)], np.int64)
    idx16, dst_t, w_t = _build_tables(
        owner, ukey1, src_local1, dloc, w_f, unit_off, len(unit_order)
    )

    # ---------------- pass 2: units = (kchunk, block) ----------------------
    kb = _kchunk_blocks(nblk)  # block bounds, len KC+1
    krow = kb * BLK  # row bounds within shard (last may exceed shard)
    krow[-1] = shard
    rows_k = np.diff(krow)  # rows per chunk per core

    assert all(r % 2 == 0 for r in rows_k), "pair trick needs even chunk rows"
    sp_core = src_pos // shard
    sp_local = src_pos - sp_core * shard
    sp_blk = sp_local // BLK
    kc_of = np.searchsorted(kb[1:], sp_blk, side="right")
    crow = sp_core * rows_k[kc_of] + (sp_local - krow[kc_of])  # row in y2f[k]
    idxval2 = crow // 2
    par2 = (crow % 2).astype(np.float32)

    cnt2 = np.zeros((n_cores, KC, nblk), np.int64)
    np.add.at(cnt2, (owner, kc_of, b_of), 1)
    gp2 = -(-cnt2.max(axis=0) // BLK)  # [KC, nblk]
    gp2 = np.maximum(gp2, 1)

    unit_order2 = [(k, b) for k in range(KC) for b in range(nblk)]
    unit_sizes2 = np.array([gp2[k, b] for k, b in unit_order2], np.int64)
    unit_off2 = np.concatenate([[0], np.cumsum(unit_sizes2)])
    ngroups2 = int(unit_off2[-1])

    blk_of_g2 = np.zeros(ngroups2, np.int64)
    for i, (k, b) in enumerate(unit_order2):
        blk_of_g2[unit_off2[i] : unit_off2[i + 1]] = b
    # first/last group per (block, k)
    fc2 = np.zeros((nblk, KC), np.int64)
    lc2 = np.zeros((nblk, KC), np.int64)
    for i, (k, b) in enumerate(unit_order2):
        fc2[b, k] = unit_off2[i]
        lc2[b, k] = unit_off2[i + 1] - 1

    chunks2 = []
    for k in range(KC):
        g0 = int(unit_off2[k * nblk])
        g1 = int(unit_off2[(k + 1) * nblk])
        for s in range(g0, g1, MAX_G_CHUNK):
            chunks2.append((s, min(s + MAX_G_CHUNK, g1), k))

    ukey2 = kc_of * nblk + b_of
    idx16c, dst2_t, w2_t = _build_tables(
        owner, ukey2, idxval2, dloc + BLK * par2, w_f, unit_off2, KC * nblk
    )

    plan = {
        "chunks": chunks,
        "chunks2": chunks2,
        "blk_of_g": [int(x) for x in blk_of_g],
        "first_g": [int(x) for x in first_g],
        "last_g": [int(x) for x in last_g],
        "blk_of_g2": [int(x) for x in blk_of_g2],
        "fc2": fc2,
        "lc2": lc2,
        "kb": [int(x) for x in kb],
        "rows_k": [int(x) for x in rows_k],
        "nblk": nblk,
        "ngroups": ngroups,
        "ngroups2": ngroups2,
        "pos": pos,
    }
    return idx16, idx16c, dst_t, dst2_t, w_t, w2_t, plan


def _build(n_nodes, hid, plan, n_cores, n_queues=4):
    """Build the SPMD Bass program from the edge plan."""
    shard = n_nodes // n_cores
    nblk = plan["nblk"]
    ngroups = plan["ngroups"]
    ngroups2 = plan["ngroups2"]
    chunks = plan["chunks"]
    chunks2 = plan["chunks2"]
    blk_of_g = plan["blk_of_g"]
    first_g = plan["first_g"]
    last_g = plan["last_g"]
    blk_of_g2 = plan["blk_of_g2"]
    fc2 = plan["fc2"]
    lc2 = plan["lc2"]
    kb = plan["kb"]
    rows_k = plan["rows_k"]
    split = n_nodes // 2
    h2 = 2 * hid

    nc = bacc.Bacc(
        None,
        num_devices=n_cores,
        num_swdge_queues=n_queues,
        dynamic_dma_scratch_size=16 * BLK * MAX_G_CHUNK,
    )

    x1b = nc.dram_tensor("x1b", [n_nodes, h2], BF16, kind="ExternalInput")
    state_s = nc.dram_tensor("state_s", [shard, hid], F32, kind="ExternalInput")
    featT_s = nc.dram_tensor("featT_s", [hid, shard], BF16, kind="ExternalInput")
    stateT_s = nc.dram_tensor("stateT_s", [hid, shard], BF16, kind="ExternalInput")
    idx16_d = nc.dram_tensor("idx16", [BLK, 8 * ngroups], I16, kind="ExternalInput")
    idx2_d = nc.dram_tensor("idx2", [BLK, 8 * ngroups2], I16, kind="ExternalInput")
    dst_d = nc.dram_tensor("dst_t", [BLK, ngroups], F32, kind="ExternalInput")
    dst2_d = nc.dram_tensor("dst2_t", [BLK, ngroups2], F32, kind="ExternalInput")
    w_d = nc.dram_tensor("w_t", [BLK, ngroups], F32, kind="ExternalInput")
    w2_d = nc.dram_tensor("w2_t", [BLK, ngroups2], F32, kind="ExternalInput")
    wzr = nc.dram_tensor("wzr", [h2, h2], F32, kind="ExternalInput")
    bzr = nc.dram_tensor("bzr", [1, h2], F32, kind="ExternalInput")
    wc = nc.dram_tensor("wc", [h2, hid], F32, kind="ExternalInput")
    bc = nc.dram_tensor("bc", [1, hid], F32, kind="ExternalInput")
    out = nc.dram_tensor("out", [shard, hid], F32, kind="ExternalOutput")

    y2s = [
        nc.dram_tensor(f"y2s{k}", [rows_k[k], hid], BF16, kind="Internal")
        for k in range(KC)
    ]
    y2f = [
        nc.dram_tensor(
            f"y2f{k}", [n_cores * rows_k[k], hid], BF16, kind="Internal",
            addr_space="Shared",
        )
        for k in range(KC)
    ]

    mx1 = max(g1 - g0 for g0, g1, _ in chunks)
    mx2 = max(g1 - g0 for g0, g1, _ in chunks2)
    qn = [0]

    def next_q():
        q = qn[0]
        qn[0] = (qn[0] + 1) % n_queues
        return q

    def rows_of(b):
        return BLK if b < nblk - 1 else shard - (nblk - 1) * BLK

    with tile.TileContext(nc) as tc:
        with (
            tc.tile_pool(name="const", bufs=1) as const_pool,
            tc.tile_pool(name="store", bufs=1) as store_pool,
            tc.tile_pool(name="msg", bufs=2) as msg_pool,
            tc.tile_pool(name="oh", bufs=6) as oh_pool,
            tc.tile_pool(name="blk", bufs=6) as blk_pool,
            tc.tile_pool(name="agg_ps", bufs=SB_BLOCKS, space="PSUM") as agg_psum,
            tc.tile_pool(name="mm_ps", bufs=2, space="PSUM") as mm_psum,
        ):
            nc.gpsimd.load_library(mlp)
            # ---- constants ----
            iota_i = const_pool.tile([BLK, BLK], mybir.dt.int32)
            nc.gpsimd.iota(iota_i[:], pattern=[[1, BLK]], base=0, channel_multiplier=0)
            iota_h = const_pool.tile([BLK, BLK], BF16)
            nc.vector.tensor_copy(iota_h[:], iota_i[:])
            iota2_i = const_pool.tile([BLK, 2 * BLK], mybir.dt.int32)
            nc.gpsimd.iota(
                iota2_i[:], pattern=[[1, 2 * BLK]], base=0, channel_multiplier=0
            )
            iota2_h = const_pool.tile([BLK, 2 * BLK], BF16)
            nc.vector.tensor_copy(iota2_h[:], iota2_i[:])
            ones1 = const_pool.tile([1, BLK], F32)
            nc.vector.memset(ones1[:], 1.0)
            wzr_sb = const_pool.tile([h2, h2], F32)
            nc.sync.dma_start(out=wzr_sb[:], in_=wzr[:, :])
            bzr_sb = const_pool.tile([1, h2], F32)
            nc.sync.dma_start(out=bzr_sb[:], in_=bzr[:, :])
            wct_f32 = const_pool.tile([hid, hid], F32)
            nc.sync.dma_start(out=wct_f32[:], in_=wc[0:hid, :])
            wcb_f32 = const_pool.tile([hid, hid], F32)
            nc.sync.dma_start(out=wcb_f32[:], in_=wc[hid:h2, :])
            wctop_sb = const_pool.tile([hid, hid], BF16)
            nc.vector.tensor_copy(wctop_sb[:], wct_f32[:])
            wcbot_sb = const_pool.tile([hid, hid], BF16)
            nc.vector.tensor_copy(wcbot_sb[:], wcb_f32[:])
            bc_sb = const_pool.tile([1, hid], F32)
            nc.sync.dma_start(out=bc_sb[:], in_=bc[:, :])

            # ---- persistent stores ----
            idx16_sb = store_pool.tile([BLK, 8 * ngroups], I16)
            nc.sync.dma_start(out=idx16_sb[:], in_=idx16_d[:, :])
            idx2_sb = store_pool.tile([BLK, 8 * ngroups2], I16)
            nc.sync.dma_start(out=idx2_sb[:], in_=idx2_d[:, :])
            dst_sb = store_pool.tile([BLK, ngroups], F32)
            nc.sync.dma_start(out=dst_sb[:], in_=dst_d[:, :])
            w_sb = store_pool.tile([BLK, ngroups], F32)
            nc.sync.dma_start(out=w_sb[:], in_=w_d[:, :])
            dst2_sb = store_pool.tile([BLK, ngroups2], F32)
            nc.sync.dma_start(out=dst2_sb[:], in_=dst2_d[:, :])
            w2_sb = store_pool.tile([BLK, ngroups2], F32)
            nc.sync.dma_start(out=w2_sb[:], in_=w2_d[:, :])

            nfull = (nblk - 1) * BLK  # rows in full blocks
            featT_store = store_pool.tile([hid, nblk * BLK], BF16)
            nc.vector.memset(featT_store[:], 0.0)
            nc.sync.dma_start(out=featT_store[:, 0:shard], in_=featT_s[:, :])
            stateT_store = store_pool.tile([hid, nblk * BLK], BF16)
            nc.vector.memset(stateT_store[:], 0.0)
            nc.sync.dma_start(out=stateT_store[:, 0:shard], in_=stateT_s[:, :])
            st_store = store_pool.tile([BLK, nblk * hid], F32)
            nc.vector.memset(st_store[:], 0.0)
            nc.sync.dma_start(
                out=st_store[:, 0 : (nblk - 1) * hid].rearrange(
                    "p (b h) -> p b h", h=hid
                ),
                in_=state_s[0:nfull, :].rearrange("(b p) h -> p b h", p=BLK),
            )
            nc.sync.dma_start(
                out=st_store[: shard - nfull, (nblk - 1) * hid : nblk * hid],
                in_=state_s[nfull:shard, :],
            )
            z_store = store_pool.tile([BLK, nblk * hid], F32)
            acc_store = store_pool.tile([BLK, nblk * hid], F32)

            # ============== Phase A: pass-1 aggregation + y2 ===============
            psum_of = {}
            done_blocks = [0]
            coll_emitted = [0]

            def tail_a(b):
                """Post-aggregation per-block work for pass 1."""
                R = rows_of(b)
                k = int(np.searchsorted(kb[1:], b, side="right"))
                aggT_ps = psum_of.pop(b)
                aggT = blk_pool.tile([h2, BLK], F32, tag="aggT")
                nc.vector.tensor_copy(aggT[:], aggT_ps[:])
                zr_ps = mm_psum.tile([BLK, hid], F32, tag="mm")
                nc.tensor.matmul(
                    zr_ps[:], lhsT=aggT[:], rhs=wzr_sb[:, 0:hid], start=True, stop=False
                )
                nc.tensor.matmul(
                    zr_ps[:], lhsT=ones1[:], rhs=bzr_sb[:, 0:hid], start=False, stop=True
                )
                nc.scalar.activation(
                    z_store[:, b * hid : (b + 1) * hid],
                    zr_ps[:],
                    mybir.ActivationFunctionType.Sigmoid,
                )
                rT_ps = mm_psum.tile([hid, BLK], F32, tag="mm")
                nc.tensor.matmul(
                    rT_ps[:], lhsT=wzr_sb[:, hid:h2], rhs=aggT[:], start=True, stop=False
                )
                nc.tensor.matmul(
                    rT_ps[:], lhsT=bzr_sb[:, hid:h2], rhs=ones1[:], start=False, stop=True
                )
                rT_sb = blk_pool.tile([hid, BLK], BF16, tag="rT")
                nc.scalar.activation(
                    rT_sb[:], rT_ps[:], mybir.ActivationFunctionType.Sigmoid
                )
                rsT = blk_pool.tile([hid, BLK], BF16, tag="rsT")
                nc.vector.tensor_tensor(
                    out=rsT[:],
                    in0=rT_sb[:],
                    in1=stateT_store[:, b * BLK : (b + 1) * BLK],
                    op=mybir.AluOpType.mult,
                )
                y2_ps = mm_psum.tile([BLK, hid], F32, tag="mm")
                nc.tensor.matmul(
                    y2_ps[:],
                    lhsT=featT_store[:, b * BLK : (b + 1) * BLK],
                    rhs=wctop_sb[:],
                    start=True,
                    stop=False,
                )
                nc.tensor.matmul(
                    y2_ps[:], lhsT=rsT[:], rhs=wcbot_sb[:], start=False, stop=True
                )
                y2_sb = blk_pool.tile([BLK, hid], BF16, tag="y2")
                nc.vector.tensor_copy(y2_sb[:], y2_ps[:])
                r0 = b * BLK - kb[k] * BLK  # row offset within source chunk k
                nc.sync.dma_start(out=y2s[k][r0 : r0 + R, :], in_=y2_sb[:R, :])
                done_blocks[0] += 1

            def maybe_emit_colls():
                while coll_emitted[0] < KC and done_blocks[0] >= kb[coll_emitted[0] + 1]:
                    k = coll_emitted[0]
                    nc.gpsimd.collective_compute(
                        "AllGather",
                        mybir.AluOpType.bypass,
                        replica_groups=[list(range(n_cores))],
                        ins=[y2s[k][:, :]],
                        outs=[y2f[k][:, :]],
                    )
                    coll_emitted[0] += 1

            for g0, g1, c in chunks:
                kg = g1 - g0
                nidx = kg * BLK
                tbl = x1b[0:split, :] if c == 0 else x1b[split:n_nodes, :]
                msgs = msg_pool.tile([BLK, max(mx1, mx2) * h2], BF16, tag="m1")
                out_ap = msgs[:, : kg * h2].rearrange("p (t w) -> p t w", w=h2)
                nc.gpsimd.dma_gather(
                    out_ap,
                    tbl,
                    idx16_sb[:, 8 * g0 : 8 * g1],
                    nidx,
                    nidx,
                    h2,
                    queue_num=next_q(),
                    single_packet=False,
                )
                for g in range(g0, g1):
                    b = blk_of_g[g]
                    if b not in psum_of:
                        psum_of[b] = agg_psum.tile(
                            [h2, BLK], F32, tag="agg", name=f"agga{b}"
                        )
                    oh = oh_pool.tile([BLK, BLK], BF16, tag="oh")
                    nc.vector.tensor_scalar(
                        out=oh[:],
                        in0=iota_h[:],
                        scalar1=dst_sb[:, g : g + 1],
                        scalar2=w_sb[:, g : g + 1],
                        op0=mybir.AluOpType.is_equal,
                        op1=mybir.AluOpType.mult,
                    )
                    gl = (g - g0) * h2
                    nc.tensor.matmul(
                        out=psum_of[b][:],
                        lhsT=msgs[:, gl : gl + h2],
                        rhs=oh[:],
                        start=(g == first_g[b]),
                        stop=(g == last_g[b]),
                    )
                    if g == last_g[b]:
                        tail_a(b)
                maybe_emit_colls()

            # ============== Phase C: pass-2 sweeps over source chunks =======
            def acc_c(b, k, psum_c):
                """Fold sweep-k partial aggregate for block b into SBUF/output."""
                R = rows_of(b)
                sl = slice(b * hid, (b + 1) * hid)
                if k == 0:
                    nc.vector.tensor_copy(acc_store[:, sl], psum_c[:])
                    return
                if k < KC - 1:
                    nc.vector.tensor_tensor(
                        out=acc_store[:, sl],
                        in0=psum_c[:],
                        in1=acc_store[:, sl],
                        op=mybir.AluOpType.add,
                    )
                    return
                t0 = blk_pool.tile([BLK, hid], F32, tag="t0")
                nc.vector.tensor_tensor(
                    out=t0[:], in0=psum_c[:], in1=acc_store[:, sl],
                    op=mybir.AluOpType.add,
                )
                c_sb = blk_pool.tile([BLK, hid], F32, tag="c")
                nc.scalar.activation(
                    c_sb[:], t0[:], mybir.ActivationFunctionType.Tanh
                )
                # new_state = c + z*(state - c)
                t1 = blk_pool.tile([BLK, hid], F32, tag="t1")
                nc.vector.tensor_tensor(
                    out=t1[:],
                    in0=st_store[:, sl],
                    in1=c_sb[:],
                    op=mybir.AluOpType.subtract,
                )
                t2 = blk_pool.tile([BLK, hid], F32, tag="t2")
                nc.vector.tensor_tensor(
                    out=t2[:],
                    in0=t1[:],
                    in1=z_store[:, sl],
                    op=mybir.AluOpType.mult,
                )
                ns = blk_pool.tile([BLK, hid], F32, tag="ns")
                nc.vector.tensor_tensor(
                    out=ns[:], in0=t2[:], in1=c_sb[:], op=mybir.AluOpType.add
                )
                nc.sync.dma_start(out=out[b * BLK : b * BLK + R, :], in_=ns[:R, :])

            psum_c_of = {}
            for g0, g1, k in chunks2:
                kg = g1 - g0
                nidx = kg * BLK
                tbl = y2f[k][:, :].rearrange("(n two) h -> n (two h)", two=2)
                msgs2 = msg_pool.tile([BLK, max(mx1, mx2) * h2], BF16, tag="m1")
                out_ap = msgs2[:, : kg * h2].rearrange("p (t w) -> p t w", w=h2)
                nc.gpsimd.dma_gather(
                    out_ap,
                    tbl,
                    idx2_sb[:, 8 * g0 : 8 * g1],
                    nidx,
                    nidx,
                    h2,
                    queue_num=next_q(),
                    single_packet=False,
                )
                for g in range(g0, g1):
                    b = blk_of_g2[g]
                    if b not in psum_c_of:
                        psum_c_of[b] = agg_psum.tile(
                            [BLK, hid], F32, tag="agg", name=f"aggc{b}k{k}"
                        )
                    ohp = oh_pool.tile([BLK, 2 * BLK], BF16, tag="ohp")
                    nc.vector.tensor_scalar(
                        out=ohp[:],
                        in0=iota2_h[:],
                        scalar1=dst2_sb[:, g : g + 1],
                        scalar2=w2_sb[:, g : g + 1],
                        op0=mybir.AluOpType.is_equal,
                        op1=mybir.AluOpType.mult,
                    )
                    gl = (g - g0) * h2
                    last_in_sweep = g == lc2[b][k]
                    final = k == KC - 1
                    nc.tensor.matmul(
                        out=psum_c_of[b][:],
                        lhsT=ohp[:, 0:BLK],
                        rhs=msgs2[:, gl : gl + hid],
                        start=(g == fc2[b][k]),
                        stop=False,
                    )
                    nc.tensor.matmul(
                        out=psum_c_of[b][:],
                        lhsT=ohp[:, BLK : 2 * BLK],
                        rhs=msgs2[:, gl + hid : gl + h2],
                        start=False,
                        stop=(last_in_sweep and not final),
                    )
                    if last_in_sweep:
                        psum_c = psum_c_of.pop(b)
                        if final:
                            nc.tensor.matmul(
                                psum_c[:], lhsT=ones1[:], rhs=bc_sb[:],
                                start=False, stop=True,
                            )
                        acc_c(b, k, psum_c)

    nc.finalize()
    return nc


def run(feat, state, src, dst, edge_weight, Wzr, bzr, Wc, bc, trace=False):
    """Build + run on 8 cores; returns (new_state, BassKernelResults)."""
    n_nodes, hid = feat.shape
    n_cores = N_CORES
    shard = n_nodes // n_cores

    idx16, idx16c, dst_t, dst2_t, w_t, w2_t, plan = _prep_edges(
        dst, src, edge_weight, n_nodes, n_cores
    )
    import ml_dtypes

    pos = plan["pos"]
    # global permutation: node (p, l) lives at row p*shard + pos[p, l]
    inv = np.empty((n_cores, shard), np.int64)
    for p in range(n_cores):
        inv[p, pos[p]] = np.arange(shard)
    x1 = np.concatenate([feat, state], axis=1)
    x1p = np.empty_like(x1)
    for p in range(n_cores):
        x1p[p * shard : (p + 1) * shard] = x1[p * shard : (p + 1) * shard][inv[p]]
    x1b = np.ascontiguousarray(x1p.astype(ml_dtypes.bfloat16))

    nc = _build(n_nodes, hid, plan, n_cores)

    in_maps = []
    for p in range(n_cores):
        feat_p = feat[p * shard : (p + 1) * shard][inv[p]]
        state_p = state[p * shard : (p + 1) * shard][inv[p]]
        in_maps.append(
            {
                "x1b": x1b,
                "state_s": np.ascontiguousarray(state_p),
                "featT_s": np.ascontiguousarray(feat_p.T.astype(ml_dtypes.bfloat16)),
                "stateT_s": np.ascontiguousarray(state_p.T.astype(ml_dtypes.bfloat16)),
                "idx16": np.ascontiguousarray(idx16[p]),
                "idx2": np.ascontiguousarray(idx16c[p]),
                "dst_t": np.ascontiguousarray(dst_t[p]),
                "dst2_t": np.ascontiguousarray(dst2_t[p]),
                "w_t": np.ascontiguousarray(w_t[p]),
                "w2_t": np.ascontiguousarray(w2_t[p]),
                "wzr": np.ascontiguousarray(Wzr, dtype=np.float32),
                "bzr": np.ascontiguousarray(bzr.reshape(1, -1), dtype=np.float32),
                "wc": np.ascontiguousarray(Wc, dtype=np.float32),
                "bc": np.ascontiguousarray(bc.reshape(1, -1), dtype=np.float32),
            }
        )

    res = run_bass_kernel_spmd(
        nc, in_maps, core_ids=list(range(n_cores)), trace=trace
    )
    shards = [res.results[p]["out"][pos[p]] for p in range(n_cores)]
    return np.concatenate(shards, axis=0), res


def kernel(feat, state, src, dst, edge_weight, Wzr, bzr, Wc, bc):
    out, _ = run(feat, state, src, dst, edge_weight, Wzr, bzr, Wc, bc, trace=False)
    return out


# revision 27
# speedup vs baseline: 1.2626x; 1.1007x over previous
"""DCGRU cell on 8 Trainium2 NeuronCores.

Strategy (dst-sharded graph partitioning, overlapped AllGather):
  - Nodes are sharded into 8 contiguous ranges (one per core). Edges are
    assigned to the core owning dst; within a core, dst nodes are processed
    in blocks of 128 grouped into super-blocks of SB_BLOCKS for gathering.
  - Pass 1 gathers x1 = [feat, state] rows (bf16, 256B) per edge and
    segment-sums them via one-hot matmuls into PSUM (transposed layout).
    zr/r^T/y2 are produced without PE transposes: the host supplies feat^T
    and state^T so y2 = feat @ Wc_top + (r*state) @ Wc_bot uses featT/rsT
    directly as lhsT.
  - y2 (bf16) is AllGathered in KC block-range chunks, each issued as soon
    as its blocks finish in pass 1, so the collective overlaps phase-A
    compute (collective cost model: 15us fixed + bytes/40GBps).
  - Pass 2 gathers y2 rows (128B) directly, sweeping source-chunks k-outer
    so sweep k only waits on collective chunk k; per (block, sweep) partial
    PSUM aggregates accumulate into an SBUF accumulator.
"""

import numpy as np

import concourse.bass as bass
import concourse.bacc as bacc
import concourse.mybir as mybir
import concourse.tile as tile
from concourse.bass_utils import run_bass_kernel_spmd
from concourse.library_config import mlp
from concourse.masks import make_identity

N_NODES = 50000
N_EDGES = 640000
HID = 64
N_CORES = 8
BLK = 128           # dst nodes per block (= PSUM partition dim)
SB_BLOCKS = 4       # dst blocks per super-block (PSUM live tiles = SB_BLOCKS)
MAX_G_CHUNK = 32    # cap on groups per dma_gather instruction
KC = 4              # source chunks for the AllGather / pass-2 sweeps

F32 = mybir.dt.float32
BF16 = mybir.dt.bfloat16
I16 = mybir.dt.int16


def _kchunk_blocks(nblk):
    """Block ranges per source chunk: decreasing sizes so each sweep's
    collective chunk lands just before the sweep needs it."""
    if nblk == 49 and KC == 4:
        sizes = [18, 16, 10, 5]
    else:
        base = nblk // KC
        rem = nblk % KC
        sizes = [base + (1 if i < rem else 0) for i in range(KC)]
    bounds = np.concatenate([[0], np.cumsum(sizes)])
    return bounds  # len KC+1, bounds[-1] == nblk


def _build_tables(e_owner, e_ukey, e_idxval, e_dloc, e_w, unit_off, n_units):
    """Slot edges into (group, lane) and build idx16/dst/w tables.

    e_* are per-edge arrays; e_ukey is the unit id (dense, 0..n_units-1);
    e_idxval is the int16 gather index value. Returns (idx16, dst_t, w_t).
    """
    e = len(e_owner)
    ngroups = int(unit_off[-1])
    ck = e_owner * n_units + e_ukey
    order = np.argsort(ck, kind="stable")
    ck_s = ck[order]
    owner_s = e_owner[order]
    idxval_s = e_idxval[order]
    ukey_s = e_ukey[order]
    dloc_s = e_dloc[order]
    w_s = e_w[order]
    bucket_start = np.searchsorted(ck_s, np.arange(N_CORES * n_units))
    rank = np.arange(e) - bucket_start[ck_s]
    g_global = unit_off[ukey_s] + rank // BLK
    lane = rank % BLK

    idx16 = np.zeros((N_CORES, 16, 8 * ngroups), np.int16)
    dst_t = np.zeros((N_CORES, BLK, ngroups), np.float32)
    w_t = np.zeros((N_CORES, BLK, ngroups), np.float32)
    idx16[owner_s, lane % 16, 8 * g_global + lane // 16] = idxval_s.astype(np.int16)
    dst_t[owner_s, lane, g_global] = dloc_s
    w_t[owner_s, lane, g_global] = w_s
    idx16 = np.tile(idx16, (1, 8, 1))
    return idx16, dst_t, w_t


def _prep_edges(dst, src, edge_weight, n_nodes, n_cores):
    """Partition edges by dst core/block; build pass-1 and pass-2 tables."""
    shard = n_nodes // n_cores
    nblk = (shard + BLK - 1) // BLK
    split = n_nodes // 2
    e = len(dst)

    dsts = dst.astype(np.int64)
    srcs = src.astype(np.int64)
    owner = dsts // shard
    local = dsts - owner * shard

    # Balance in-degree across blocks: per core, deal nodes (sorted by
    # in-degree, desc) round-robin over blocks. pos[core, orig_local] is the
    # node's new row; node tables / shard rows / outputs use this order.
    deg = np.zeros(n_nodes, np.int64)
    np.add.at(deg, dsts, 1)
    pos = np.empty((n_cores, shard), np.int64)
    blk_fill = np.empty(nblk, np.int64)
    cap = np.full(nblk, BLK, np.int64)
    cap[nblk - 1] = shard - (nblk - 1) * BLK
    for p in range(n_cores):
        nodes = np.argsort(-deg[p * shard : (p + 1) * shard], kind="stable")
        blk_fill[:] = 0
        bi = 0
        for n in nodes:
            while blk_fill[bi % nblk] >= cap[bi % nblk]:
                bi += 1
            b = bi % nblk
            pos[p, n] = b * BLK + blk_fill[b]
            blk_fill[b] += 1
            bi += 1
    newloc = pos[owner, local]
    b_of = newloc // BLK
    dloc = (newloc % BLK).astype(np.float32)
    w_f = edge_weight.astype(np.float32)

    # permuted global src position
    src_pos = pos[srcs // shard, srcs % shard] + (srcs // shard) * shard

    # ---------------- pass 1: units = (super-block, class, block) ----------
    cls = (src_pos >= split).astype(np.int64)
    src_local1 = src_pos - cls * split

    cnt = np.zeros((n_cores, nblk, 2), np.int64)
    np.add.at(cnt, (owner, b_of, cls), 1)
    gpbc = -(-cnt.max(axis=0) // BLK)  # [nblk, 2]
    empty = gpbc.sum(axis=1) == 0
    gpbc[empty, 0] = 1

    unit_order = []
    for sb0 in range(0, nblk, SB_BLOCKS):
        sbb = range(sb0, min(sb0 + SB_BLOCKS, nblk))
        for c in range(2):
            for b in sbb:
                unit_order.append((b, c))
    unit_sizes = np.array([gpbc[b, c] for b, c in unit_order], np.int64)
    unit_off = np.concatenate([[0], np.cumsum(unit_sizes)])
    ngroups = int(unit_off[-1])
    unit_idx = {bc: i for i, bc in enumerate(unit_order)}

    blk_of_g = np.zeros(ngroups, np.int64)
    for i, (b, c) in enumerate(unit_order):
        blk_of_g[unit_off[i] : unit_off[i + 1]] = b
    first_g = np.full(nblk, -1, np.int64)
    last_g = np.full(nblk, -1, np.int64)
    for g in range(ngroups):
        b = blk_of_g[g]
        if first_g[b] < 0:
            first_g[b] = g
        last_g[b] = g

    # gather chunks: contiguous same-class unit runs within a super-block
    chunks = []
    i = 0
    while i < len(unit_order):
        c = unit_order[i][1]
        sb = unit_order[i][0] // SB_BLOCKS
        j = i
        while (
            j < len(unit_order)
            and unit_order[j][1] == c
            and unit_order[j][0] // SB_BLOCKS == sb
        ):
            j += 1
        g0, g1 = int(unit_off[i]), int(unit_off[j])
        for s in range(g0, g1, MAX_G_CHUNK):
            if s < g1:
                chunks.append((s, min(s + MAX_G_CHUNK, g1), c))
        i = j

    ukey1 = np.array([unit_idx[(b, c)] for b, c in zip(b_of, cls)], np.int64)
    idx16, dst_t, w_t = _build_tables(
        owner, ukey1, src_local1, dloc, w_f, unit_off, len(unit_order)
    )

    # ---------------- pass 2: units = (kchunk, block) ----------------------
    kb = _kchunk_blocks(nblk)  # block bounds, len KC+1
    krow = kb * BLK  # row bounds within shard (last may exceed shard)
    krow[-1] = shard
    rows_k = np.diff(krow)  # rows per chunk per core

    assert all(r % 2 == 0 for r in rows_k), "pair trick needs even chunk rows"
    sp_core = src_pos // shard
    sp_local = src_pos - sp_core * shard
    sp_blk = sp_local // BLK
    kc_of = np.searchsorted(kb[1:], sp_blk, side="right")
    crow = sp_core * rows_k[kc_of] + (sp_local - krow[kc_of])  # row in y2f[k]
    idxval2 = crow // 2
    par2 = (crow % 2).astype(np.float32)

    cnt2 = np.zeros((n_cores, KC, nblk), np.int64)
    np.add.at(cnt2, (owner, kc_of, b_of), 1)
    gp2 = -(-cnt2.max(axis=0) // BLK)  # [KC, nblk]
    gp2 = np.maximum(gp2, 1)

    unit_order2 = [(k, b) for k in range(KC) for b in range(nblk)]
    unit_sizes2 = np.array([gp2[k, b] for k, b in unit_order2], np.int64)
    unit_off2 = np.concatenate([[0], np.cumsum(unit_sizes2)])
    ngroups2 = int(unit_off2[-1])

    blk_of_g2 = np.zeros(ngroups2, np.int64)
    for i, (k, b) in enumerate(unit_order2):
        blk_of_g2[unit_off2[i] : unit_off2[i + 1]] = b
    # first/last group per (block, k)
    fc2 = np.zeros((nblk, KC), np.int64)
    lc2 = np.zeros((nblk, KC), np.int64)
    for i, (k, b) in enumerate(unit_order2):
        fc2[b, k] = unit_off2[i]
        lc2[b, k] = unit_off2[i + 1] - 1

    chunks2 = []
    for k in range(KC):
        g0 = int(unit_off2[k * nblk])
        g1 = int(unit_off2[(k + 1) * nblk])
        for s in range(g0, g1, MAX_G_CHUNK):
            chunks2.append((s, min(s + MAX_G_CHUNK, g1), k))

    ukey2 = kc_of * nblk + b_of
    idx16c, dst2_t, w2_t = _build_tables(
        owner, ukey2, idxval2, dloc + BLK * par2, w_f, unit_off2, KC * nblk
    )

    plan = {
        "chunks": chunks,
        "chunks2": chunks2,
        "blk_of_g": [int(x) for x in blk_of_g],
        "first_g": [int(x) for x in first_g],
        "last_g": [int(x) for x in last_g],
        "blk_of_g2": [int(x) for x in blk_of_g2],
        "fc2": fc2,
        "lc2": lc2,
        "kb": [int(x) for x in kb],
        "rows_k": [int(x) for x in rows_k],
        "nblk": nblk,
        "ngroups": ngroups,
        "ngroups2": ngroups2,
        "pos": pos,
    }
    return idx16, idx16c, dst_t, dst2_t, w_t, w2_t, plan


def _build(n_nodes, hid, plan, n_cores, n_queues=4):
    """Build the SPMD Bass program from the edge plan."""
    shard = n_nodes // n_cores
    nblk = plan["nblk"]
    ngroups = plan["ngroups"]
    ngroups2 = plan["ngroups2"]
    chunks = plan["chunks"]
    chunks2 = plan["chunks2"]
    blk_of_g = plan["blk_of_g"]
    first_g = plan["first_g"]
    last_g = plan["last_g"]
    blk_of_g2 = plan["blk_of_g2"]
    fc2 = plan["fc2"]
    lc2 = plan["lc2"]
    kb = plan["kb"]
    rows_k = plan["rows_k"]
    split = n_nodes // 2
    h2 = 2 * hid

    nc = bacc.Bacc(
        None,
        num_devices=n_cores,
        num_swdge_queues=n_queues,
        dynamic_dma_scratch_size=16 * BLK * MAX_G_CHUNK,
    )

    x1b = nc.dram_tensor("x1b", [n_nodes, h2], BF16, kind="ExternalInput")
    state_s = nc.dram_tensor("state_s", [shard, hid], F32, kind="ExternalInput")
    featT_s = nc.dram_tensor("featT_s", [hid, shard], BF16, kind="ExternalInput")
    stateT_s = nc.dram_tensor("stateT_s", [hid, shard], BF16, kind="ExternalInput")
    idx16_d = nc.dram_tensor("idx16", [BLK, 8 * ngroups], I16, kind="ExternalInput")
    idx2_d = nc.dram_tensor("idx2", [BLK, 8 * ngroups2], I16, kind="ExternalInput")
    dst_d = nc.dram_tensor("dst_t", [BLK, ngroups], F32, kind="ExternalInput")
    dst2_d = nc.dram_tensor("dst2_t", [BLK, ngroups2], F32, kind="ExternalInput")
    w_d = nc.dram_tensor("w_t", [BLK, ngroups], F32, kind="ExternalInput")
    w2_d = nc.dram_tensor("w2_t", [BLK, ngroups2], F32, kind="ExternalInput")
    wzr = nc.dram_tensor("wzr", [h2, h2], F32, kind="ExternalInput")
    bzr = nc.dram_tensor("bzr", [1, h2], F32, kind="ExternalInput")
    wc = nc.dram_tensor("wc", [h2, hid], F32, kind="ExternalInput")
    bc = nc.dram_tensor("bc", [1, hid], F32, kind="ExternalInput")
    out = nc.dram_tensor("out", [shard, hid], F32, kind="ExternalOutput")

    y2s = [
        nc.dram_tensor(f"y2s{k}", [rows_k[k], hid], BF16, kind="Internal")
        for k in range(KC)
    ]
    y2f = [
        nc.dram_tensor(
            f"y2f{k}", [n_cores * rows_k[k], hid], BF16, kind="Internal",
            addr_space="Shared",
        )
        for k in range(KC)
    ]

    mx1 = max(g1 - g0 for g0, g1, _ in chunks)
    mx2 = max(g1 - g0 for g0, g1, _ in chunks2)
    qn = [0]

    def next_q():
        q = qn[0]
        qn[0] = (qn[0] + 1) % n_queues
        return q

    def rows_of(b):
        return BLK if b < nblk - 1 else shard - (nblk - 1) * BLK

    with tile.TileContext(nc) as tc:
        with (
            tc.tile_pool(name="const", bufs=1) as const_pool,
            tc.tile_pool(name="store", bufs=1) as store_pool,
            tc.tile_pool(name="msg", bufs=3) as msg_pool,
            tc.tile_pool(name="oh", bufs=10) as oh_pool,
            tc.tile_pool(name="blk", bufs=6) as blk_pool,
            tc.tile_pool(name="agg_ps", bufs=SB_BLOCKS + 1, space="PSUM") as agg_psum,
            tc.tile_pool(name="mm_ps", bufs=2, space="PSUM") as mm_psum,
        ):
            nc.gpsimd.load_library(mlp)
            # ---- constants ----
            iota_i = const_pool.tile([BLK, BLK], mybir.dt.int32)
            nc.gpsimd.iota(iota_i[:], pattern=[[1, BLK]], base=0, channel_multiplier=0)
            iota_h = const_pool.tile([BLK, BLK], BF16)
            nc.vector.tensor_copy(iota_h[:], iota_i[:])
            iota2_i = const_pool.tile([BLK, 2 * BLK], mybir.dt.int32)
            nc.gpsimd.iota(
                iota2_i[:], pattern=[[1, 2 * BLK]], base=0, channel_multiplier=0
            )
            iota2_h = const_pool.tile([BLK, 2 * BLK], BF16)
            nc.vector.tensor_copy(iota2_h[:], iota2_i[:])
            ones1 = const_pool.tile([1, BLK], F32)
            nc.vector.memset(ones1[:], 1.0)
            wzr_sb = const_pool.tile([h2, h2], F32)
            nc.sync.dma_start(out=wzr_sb[:], in_=wzr[:, :])
            bzr_sb = const_pool.tile([1, h2], F32)
            nc.sync.dma_start(out=bzr_sb[:], in_=bzr[:, :])
            wct_f32 = const_pool.tile([hid, hid], F32)
            nc.sync.dma_start(out=wct_f32[:], in_=wc[0:hid, :])
            wcb_f32 = const_pool.tile([hid, hid], F32)
            nc.sync.dma_start(out=wcb_f32[:], in_=wc[hid:h2, :])
            wctop_sb = const_pool.tile([hid, hid], BF16)
            nc.vector.tensor_copy(wctop_sb[:], wct_f32[:])
            wcbot_sb = const_pool.tile([hid, hid], BF16)
            nc.vector.tensor_copy(wcbot_sb[:], wcb_f32[:])
            bc_sb = const_pool.tile([1, hid], F32)
            nc.sync.dma_start(out=bc_sb[:], in_=bc[:, :])

            # ---- persistent stores ----
            idx16_sb = store_pool.tile([BLK, 8 * ngroups], I16)
            nc.sync.dma_start(out=idx16_sb[:], in_=idx16_d[:, :])
            idx2_sb = store_pool.tile([BLK, 8 * ngroups2], I16)
            nc.sync.dma_start(out=idx2_sb[:], in_=idx2_d[:, :])
            dst_sb = store_pool.tile([BLK, ngroups], F32)
            nc.sync.dma_start(out=dst_sb[:], in_=dst_d[:, :])
            w_sb = store_pool.tile([BLK, ngroups], F32)
            nc.sync.dma_start(out=w_sb[:], in_=w_d[:, :])
            dst2_sb = store_pool.tile([BLK, ngroups2], F32)
            nc.sync.dma_start(out=dst2_sb[:], in_=dst2_d[:, :])
            w2_sb = store_pool.tile([BLK, ngroups2], F32)
            nc.sync.dma_start(out=w2_sb[:], in_=w2_d[:, :])

            nfull = (nblk - 1) * BLK  # rows in full blocks
            featT_store = store_pool.tile([hid, nblk * BLK], BF16)
            nc.vector.memset(featT_store[:], 0.0)
            nc.sync.dma_start(out=featT_store[:, 0:shard], in_=featT_s[:, :])
            stateT_store = store_pool.tile([hid, nblk * BLK], BF16)
            nc.vector.memset(stateT_store[:], 0.0)
            nc.sync.dma_start(out=stateT_store[:, 0:shard], in_=stateT_s[:, :])
            st_store = store_pool.tile([BLK, nblk * hid], F32)
            nc.vector.memset(st_store[:], 0.0)
            nc.sync.dma_start(
                out=st_store[:, 0 : (nblk - 1) * hid].rearrange(
                    "p (b h) -> p b h", h=hid
                ),
                in_=state_s[0:nfull, :].rearrange("(b p) h -> p b h", p=BLK),
            )
            nc.sync.dma_start(
                out=st_store[: shard - nfull, (nblk - 1) * hid : nblk * hid],
                in_=state_s[nfull:shard, :],
            )
            z_store = store_pool.tile([BLK, nblk * hid], F32)
            acc_store = store_pool.tile([BLK, nblk * hid], F32)

            # ============== Phase A: pass-1 aggregation + y2 ===============
            psum_of = {}
            done_blocks = [0]
            coll_emitted = [0]

            def tail_a(b):
                """Post-aggregation per-block work for pass 1."""
                R = rows_of(b)
                k = int(np.searchsorted(kb[1:], b, side="right"))
                aggT_ps = psum_of.pop(b)
                aggT = blk_pool.tile([h2, BLK], F32, tag="aggT")
                nc.vector.tensor_copy(aggT[:], aggT_ps[:])
                zr_ps = mm_psum.tile([BLK, hid], F32, tag="mm")
                nc.tensor.matmul(
                    zr_ps[:], lhsT=aggT[:], rhs=wzr_sb[:, 0:hid], start=True, stop=False
                )
                nc.tensor.matmul(
                    zr_ps[:], lhsT=ones1[:], rhs=bzr_sb[:, 0:hid], start=False, stop=True
                )
                nc.scalar.activation(
                    z_store[:, b * hid : (b + 1) * hid],
                    zr_ps[:],
                    mybir.ActivationFunctionType.Sigmoid,
                )
                rT_ps = mm_psum.tile([hid, BLK], F32, tag="mm")
                nc.tensor.matmul(
                    rT_ps[:], lhsT=wzr_sb[:, hid:h2], rhs=aggT[:], start=True, stop=False
                )
                nc.tensor.matmul(
                    rT_ps[:], lhsT=bzr_sb[:, hid:h2], rhs=ones1[:], start=False, stop=True
                )
                rT_sb = blk_pool.tile([hid, BLK], BF16, tag="rT")
                nc.scalar.activation(
                    rT_sb[:], rT_ps[:], mybir.ActivationFunctionType.Sigmoid
                )
                rsT = blk_pool.tile([hid, BLK], BF16, tag="rsT")
                nc.vector.tensor_tensor(
                    out=rsT[:],
                    in0=rT_sb[:],
                    in1=stateT_store[:, b * BLK : (b + 1) * BLK],
                    op=mybir.AluOpType.mult,
                )
                y2_ps = mm_psum.tile([BLK, hid], F32, tag="mm")
                nc.tensor.matmul(
                    y2_ps[:],
                    lhsT=featT_store[:, b * BLK : (b + 1) * BLK],
                    rhs=wctop_sb[:],
                    start=True,
                    stop=False,
                )
                nc.tensor.matmul(
                    y2_ps[:], lhsT=rsT[:], rhs=wcbot_sb[:], start=False, stop=True
                )
                y2_sb = blk_pool.tile([BLK, hid], BF16, tag="y2")
                nc.vector.tensor_copy(y2_sb[:], y2_ps[:])
                r0 = b * BLK - kb[k] * BLK  # row offset within source chunk k
                nc.sync.dma_start(out=y2s[k][r0 : r0 + R, :], in_=y2_sb[:R, :])
                done_blocks[0] += 1

            def maybe_emit_colls():
                while coll_emitted[0] < KC and done_blocks[0] >= kb[coll_emitted[0] + 1]:
                    k = coll_emitted[0]
                    nc.gpsimd.collective_compute(
                        "AllGather",
                        mybir.AluOpType.bypass,
                        replica_groups=[list(range(n_cores))],
                        ins=[y2s[k][:, :]],
                        outs=[y2f[k][:, :]],
                    )
                    coll_emitted[0] += 1

            for g0, g1, c in chunks:
                kg = g1 - g0
                nidx = kg * BLK
                tbl = x1b[0:split, :] if c == 0 else x1b[split:n_nodes, :]
                msgs = msg_pool.tile([BLK, max(mx1, mx2) * h2], BF16, tag="m1")
                out_ap = msgs[:, : kg * h2].rearrange("p (t w) -> p t w", w=h2)
                nc.gpsimd.dma_gather(
                    out_ap,
                    tbl,
                    idx16_sb[:, 8 * g0 : 8 * g1],
                    nidx,
                    nidx,
                    h2,
                    queue_num=next_q(),
                    single_packet=False,
                )
                for g in range(g0, g1):
                    b = blk_of_g[g]
                    if b not in psum_of:
                        psum_of[b] = agg_psum.tile(
                            [h2, BLK], F32, tag="agg", name=f"agga{b}"
                        )
                    oh = oh_pool.tile([BLK, BLK], BF16, tag="oh")
                    nc.vector.tensor_scalar(
                        out=oh[:],
                        in0=iota_h[:],
                        scalar1=dst_sb[:, g : g + 1],
                        scalar2=w_sb[:, g : g + 1],
                        op0=mybir.AluOpType.is_equal,
                        op1=mybir.AluOpType.mult,
                    )
                    gl = (g - g0) * h2
                    nc.tensor.matmul(
                        out=psum_of[b][:],
                        lhsT=msgs[:, gl : gl + h2],
                        rhs=oh[:],
                        start=(g == first_g[b]),
                        stop=(g == last_g[b]),
                    )
                    if g == last_g[b]:
                        tail_a(b)
                maybe_emit_colls()

            # ============== Phase C: pass-2 sweeps over source chunks =======
            def acc_c(b, k, psum_c):
                """Fold sweep-k partial aggregate for block b into SBUF/output."""
                R = rows_of(b)
                sl = slice(b * hid, (b + 1) * hid)
                if k == 0:
                    nc.vector.tensor_copy(acc_store[:, sl], psum_c[:])
                    return
                if k < KC - 1:
                    nc.vector.tensor_tensor(
                        out=acc_store[:, sl],
                        in0=psum_c[:],
                        in1=acc_store[:, sl],
                        op=mybir.AluOpType.add,
                    )
                    return
                t0 = blk_pool.tile([BLK, hid], F32, tag="t0")
                nc.vector.tensor_tensor(
                    out=t0[:], in0=psum_c[:], in1=acc_store[:, sl],
                    op=mybir.AluOpType.add,
                )
                c_sb = blk_pool.tile([BLK, hid], F32, tag="c")
                nc.scalar.activation(
                    c_sb[:], t0[:], mybir.ActivationFunctionType.Tanh
                )
                # new_state = c + z*(state - c)
                t1 = blk_pool.tile([BLK, hid], F32, tag="t1")
                nc.vector.tensor_tensor(
                    out=t1[:],
                    in0=st_store[:, sl],
                    in1=c_sb[:],
                    op=mybir.AluOpType.subtract,
                )
                t2 = blk_pool.tile([BLK, hid], F32, tag="t2")
                nc.vector.tensor_tensor(
                    out=t2[:],
                    in0=t1[:],
                    in1=z_store[:, sl],
                    op=mybir.AluOpType.mult,
                )
                ns = blk_pool.tile([BLK, hid], F32, tag="ns")
                nc.vector.tensor_tensor(
                    out=ns[:], in0=t2[:], in1=c_sb[:], op=mybir.AluOpType.add
                )
                nc.sync.dma_start(out=out[b * BLK : b * BLK + R, :], in_=ns[:R, :])

            psum_c_of = {}
            for g0, g1, k in chunks2:
                kg = g1 - g0
                nidx = kg * BLK
                tbl = y2f[k][:, :].rearrange("(n two) h -> n (two h)", two=2)
                msgs2 = msg_pool.tile([BLK, max(mx1, mx2) * h2], BF16, tag="m1")
                out_ap = msgs2[:, : kg * h2].rearrange("p (t w) -> p t w", w=h2)
                nc.gpsimd.dma_gather(
                    out_ap,
                    tbl,
                    idx2_sb[:, 8 * g0 : 8 * g1],
                    nidx,
                    nidx,
                    h2,
                    queue_num=next_q(),
                    single_packet=False,
                )
                for g in range(g0, g1):
                    b = blk_of_g2[g]
                    if b not in psum_c_of:
                        psum_c_of[b] = agg_psum.tile(
                            [BLK, hid], F32, tag="agg", name=f"aggc{b}k{k}"
                        )
                    ohp = oh_pool.tile([BLK, 2 * BLK], BF16, tag="ohp")
                    nc.vector.tensor_scalar(
                        out=ohp[:],
                        in0=iota2_h[:],
                        scalar1=dst2_sb[:, g : g + 1],
                        scalar2=w2_sb[:, g : g + 1],
                        op0=mybir.AluOpType.is_equal,
                        op1=mybir.AluOpType.mult,
                    )
                    gl = (g - g0) * h2
                    last_in_sweep = g == lc2[b][k]
                    final = k == KC - 1
                    nc.tensor.matmul(
                        out=psum_c_of[b][:],
                        lhsT=ohp[:, 0:BLK],
                        rhs=msgs2[:, gl : gl + hid],
                        start=(g == fc2[b][k]),
                        stop=False,
                    )
                    nc.tensor.matmul(
                        out=psum_c_of[b][:],
                        lhsT=ohp[:, BLK : 2 * BLK],
                        rhs=msgs2[:, gl + hid : gl + h2],
                        start=False,
                        stop=(last_in_sweep and not final),
                    )
                    if last_in_sweep:
                        psum_c = psum_c_of.pop(b)
                        if final:
                            nc.tensor.matmul(
                                psum_c[:], lhsT=ones1[:], rhs=bc_sb[:],
                                start=False, stop=True,
                            )
                        acc_c(b, k, psum_c)

    nc.finalize()
    return nc


def run(feat, state, src, dst, edge_weight, Wzr, bzr, Wc, bc, trace=False):
    """Build + run on 8 cores; returns (new_state, BassKernelResults)."""
    n_nodes, hid = feat.shape
    n_cores = N_CORES
    shard = n_nodes // n_cores

    idx16, idx16c, dst_t, dst2_t, w_t, w2_t, plan = _prep_edges(
        dst, src, edge_weight, n_nodes, n_cores
    )
    import ml_dtypes

    pos = plan["pos"]
    # global permutation: node (p, l) lives at row p*shard + pos[p, l]
    inv = np.empty((n_cores, shard), np.int64)
    for p in range(n_cores):
        inv[p, pos[p]] = np.arange(shard)
    x1 = np.concatenate([feat, state], axis=1)
    x1p = np.empty_like(x1)
    for p in range(n_cores):
        x1p[p * shard : (p + 1) * shard] = x1[p * shard : (p + 1) * shard][inv[p]]
    x1b = np.ascontiguousarray(x1p.astype(ml_dtypes.bfloat16))

    nc = _build(n_nodes, hid, plan, n_cores)

    in_maps = []
    for p in range(n_cores):
        feat_p = feat[p * shard : (p + 1) * shard][inv[p]]
        state_p = state[p * shard : (p + 1) * shard][inv[p]]
        in_maps.append(
            {
                "x1b": x1b,
                "state_s": np.ascontiguousarray(state_p),
                "featT_s": np.ascontiguousarray(feat_p.T.astype(ml_dtypes.bfloat16)),
                "stateT_s": np.ascontiguousarray(state_p.T.astype(ml_dtypes.bfloat16)),
                "idx16": np.ascontiguousarray(idx16[p]),
                "idx2": np.ascontiguousarray(idx16c[p]),
                "dst_t": np.ascontiguousarray(dst_t[p]),
                "dst2_t": np.ascontiguousarray(dst2_t[p]),
                "w_t": np.ascontiguousarray(w_t[p]),
                "w2_t": np.ascontiguousarray(w2_t[p]),
                "wzr": np.ascontiguousarray(Wzr, dtype=np.float32),
                "bzr": np.ascontiguousarray(bzr.reshape(1, -1), dtype=np.float32),
                "wc": np.ascontiguousarray(Wc, dtype=np.float32),
                "bc": np.ascontiguousarray(bc.reshape(1, -1), dtype=np.float32),
            }
        )

    res = run_bass_kernel_spmd(
        nc, in_maps, core_ids=list(range(n_cores)), trace=trace
    )
    shards = [res.results[p]["out"][pos[p]] for p in range(n_cores)]
    return np.concatenate(shards, axis=0), res


def kernel(feat, state, src, dst, edge_weight, Wzr, bzr, Wc, bc):
    out, _ = run(feat, state, src, dst, edge_weight, Wzr, bzr, Wc, bc, trace=False)
    return out
